# revision 5
# baseline (speedup 1.0000x reference)
"""Trainium2 Bass kernel for nn_DominantNeigh (GCN autoencoder, DOMINANT-style).

Distribution: nodes row-sharded across 8 NeuronCores (2048 rows/core); edges
partitioned by destination row. Per GCN layer each core computes its support
shard (h @ W in bf16 on the PE), all-gathers support into a DRAM buffer
(p-major row permutation so the SBUF->DRAM store is one contiguous DMA), then
edge messages are fetched with dma_gather (128 edges land on 128 partitions)
and the weighted segment-sum is a PE matmul against host-precomputed one-hot
x weight scatter matrices, accumulated in PSUM in 32-row dst windows (4
windows per PSUM tile via col-group tile_position). Residual + L2 row-norm +
bias + relu run on DVE/ACT from PSUM. a_hat = s @ s.T is row-sharded
([2048,16384] f32 per core, the dominant HBM write); x_hat via attr decoder.
"""

import sys

sys.path.insert(0, "/opt/trn_rl_repo")

import numpy as np
import ml_dtypes

from concourse import bacc, bass, tile
from concourse import mybir
from concourse.bass_utils import run_bass_kernel_spmd

# problem shape (hardcoded; kernel.py must be self-contained)
N, FEAT, HID, E = 16384, 512, 128, 524288
BETA, EPS = 0.001, 1e-12
NCORES = 8
NLOC = N // NCORES  # 2048
RW = 32  # dst rows per window (one 32-col PE group)
NWIN = NLOC // RW  # 64 windows per core
WPQ = 4  # windows per PSUM quad tile (128 rows)
NQ = NWIN // WPQ  # 16 quads per core
JJ = NLOC // 128  # 16 sub-rows per partition in p-major shard layout

BF = mybir.dt.bfloat16
F32 = mybir.dt.float32
I16 = mybir.dt.int16
BF_NP = ml_dtypes.bfloat16
AF = mybir.ActivationFunctionType
OP = mybir.AluOpType


def prep_graph(edge_index, edge_weight):
    """Host-side: dedup (dst,src) edges, partition by dst core/window, sort by
    gather address, build per-core scatter matrices M and gather indices.

    Returns dict with T_w [NWIN], TB [NWIN+1], T_total, M [NCORES,128,Tt,RW]
    f32, idx16 [NCORES,128,Tt*8] int16 (replicated over partition groups).
    """
    dst = np.asarray(edge_index[0], dtype=np.int64)
    src = np.asarray(edge_index[1], dtype=np.int64)
    w = np.asarray(edge_weight, dtype=np.float64) * (1.0 - BETA)

    # dedup (dst, src): segment_sum adds duplicate edges
    key = dst * N + src
    order = np.argsort(key, kind="stable")
    ks, ws = key[order], w[order]
    newgrp = np.ones(len(ks), bool)
    newgrp[1:] = ks[1:] != ks[:-1]
    gid = np.cumsum(newgrp) - 1
    wsum = np.zeros(int(gid[-1]) + 1)
    np.add.at(wsum, gid, ws)
    ukey = ks[newgrp]
    udst, usrc = ukey // N, ukey % N
    # p-major permuted row of src in the all-gathered buffer:
    # shard row r = (n%128)*JJ + n//128 for local n
    sloc = usrc % NLOC
    uv = (usrc // NLOC) * NLOC + (sloc % 128) * JJ + sloc // 128
    core = udst // NLOC
    dloc = udst % NLOC
    winloc = dloc // RW
    roff = dloc % RW

    # order by (core, window, gather address) - address order gives HBM locality
    o2 = np.lexsort((uv, winloc, core))
    core, winloc, roff, uv, wsum = core[o2], winloc[o2], roff[o2], uv[o2], wsum[o2]

    counts = np.zeros((NCORES, NWIN), np.int64)
    np.add.at(counts, (core, winloc), 1)
    T_w = np.maximum(1, -(-counts.max(axis=0) // 128))  # tiles per window
    TB = np.zeros(NWIN + 1, np.int64)
    TB[1:] = np.cumsum(T_w)
    T_total = int(TB[-1])

    # position of each entry within its (core, window) group
    gkey = core * NWIN + winloc
    newg = np.ones(len(gkey), bool)
    newg[1:] = gkey[1:] != gkey[:-1]
    gidx = np.cumsum(newg) - 1
    starts = np.flatnonzero(newg)
    pos = np.arange(len(gkey)) - starts[gidx]

    q = winloc // WPQ
    slot = (TB[winloc] - TB[WPQ * q]) * 128 + pos  # call-local gather slot
    tile_g = TB[winloc] + pos // 128
    part = pos % 128

    M = np.zeros((NCORES, 128, T_total, RW), np.float32)
    M[core, part, tile_g, roff] = wsum

    idx16 = np.zeros((NCORES, 16, T_total * 8), np.int16)
    colv = TB[WPQ * q] * 8 + slot // 16
    idx16[core, slot % 16, colv] = uv.astype(np.int16)
    idx16 = np.tile(idx16, (1, 8, 1))  # replicate to 128 partitions

    return dict(T_w=T_w, TB=TB, T_total=T_total, M=M, idx16=idx16)


def build_program(T_w, TB, T_total):
    """Build the SPMD Bass program (shared by all 8 cores)."""
    T_quad = [int(TB[WPQ * (q + 1)] - TB[WPQ * q]) for q in range(NQ)]
    Tq_max = max(T_quad)

    nc = bacc.Bacc(num_devices=NCORES)

    xt_in = nc.dram_tensor("xt", [128, 4, NLOC], BF, kind="ExternalInput")
    m_in = nc.dram_tensor("mt", [128, T_total * RW], BF, kind="ExternalInput")
    idx_in = nc.dram_tensor("idx", [128, T_total * 8], I16, kind="ExternalInput")
    encw_in = nc.dram_tensor("encw", [128, 4, HID], BF, kind="ExternalInput")
    encb_in = nc.dram_tensor("encb", [128, HID], F32, kind="ExternalInput")
    gcw_in = [
        nc.dram_tensor(f"gcw{l}", [HID, HID], BF, kind="ExternalInput")
        for l in range(5)
    ]
    gcb_in = [
        nc.dram_tensor(f"gcb{l}", [128, HID], F32, kind="ExternalInput")
        for l in range(5)
    ]
    attrw_in = nc.dram_tensor("attrw", [HID, FEAT], BF, kind="ExternalInput")
    attrb_in = nc.dram_tensor("attrb", [128, FEAT], F32, kind="ExternalInput")
    ident_in = nc.dram_tensor("ident", [128, 128], BF, kind="ExternalInput")
    ahat_out = nc.dram_tensor("ahat", [NLOC, N], F32, kind="ExternalOutput")
    xhat_out = nc.dram_tensor("xhat", [NLOC, FEAT], F32, kind="ExternalOutput")

    with tile.TileContext(nc) as tc:
        with (
            tc.tile_pool(name="const", bufs=1) as constp,
            tc.tile_pool(name="big", bufs=1) as bigp,
            tc.tile_pool(name="hp", bufs=48) as hp,
            tc.tile_pool(name="htp", bufs=4) as htp,
            tc.tile_pool(name="msgsp", bufs=2) as msgsp,
            tc.tile_pool(name="workp", bufs=4) as workp,
            tc.tile_pool(name="smallp", bufs=8) as smallp,
            tc.tile_pool(name="suppp", bufs=2) as suppp,
            tc.tile_pool(name="stagep", bufs=2) as stagep,
            tc.tile_pool(name="psum", bufs=1, space="PSUM") as psump,
            tc.tile_pool(name="dramp", bufs=2, space="DRAM") as dramp,
        ):
            # ---- constants ----
            ident = constp.tile([128, 128], BF)
            nc.sync.dma_start(ident[:], ident_in[:])
            encw = constp.tile([128, 4, HID], BF)
            nc.sync.dma_start(encw[:], encw_in[:])
            encb = constp.tile([128, HID], F32)
            nc.sync.dma_start(encb[:], encb_in[:])
            gcw, gcb = [], []
            for l in range(5):
                wt = constp.tile([HID, HID], BF, name=f"gcw{l}")
                nc.sync.dma_start(wt[:], gcw_in[l][:])
                gcw.append(wt)
                bt = constp.tile([128, HID], F32, name=f"gcb{l}")
                nc.sync.dma_start(bt[:], gcb_in[l][:])
                gcb.append(bt)
            attrw = constp.tile([HID, FEAT], BF)
            nc.sync.dma_start(attrw[:], attrw_in[:])
            attrb = constp.tile([128, FEAT], F32)
            nc.sync.dma_start(attrb[:], attrb_in[:])
            m_sb = constp.tile([128, T_total, RW], BF)
            nc.sync.dma_start(m_sb[:], m_in[:].rearrange("p (t r) -> p t r", r=RW))
            idx_sb = constp.tile([128, T_total * 8], I16)
            nc.sync.dma_start(idx_sb[:], idx_in[:])
            xt = bigp.tile([128, 4, NLOC], BF, tag="big")
            nc.sync.dma_start(xt[:], xt_in[:])

            # cached scalar registers for dma_gather num_idxs
            reg_cache = {}

            def gs_reg(v):
                if v not in reg_cache:
                    reg_cache[v] = nc.gpsimd.to_reg(v)
                return reg_cache[v]

            def new_hgen(nm):
                return [
                    hp.tile([128, 128], F32, name=f"h_{nm}_{j}", tag="h")
                    for j in range(NQ)
                ]

            # ---- encoder linear: h0 = x @ encW + encb (node-major) ----
            h0 = new_hgen("h0")
            hT0 = htp.tile([128, NLOC], BF, name="hT0", tag="ht")
            for j in range(NQ):
                ps = psump.tile([128, HID], F32, name="encps", tag="mm512", bufs=3)
                for k in range(4):
                    nc.tensor.matmul(
                        ps[:],
                        xt[:, k, 128 * j : 128 * (j + 1)],
                        encw[:, k, :],
                        start=(k == 0),
                        stop=(k == 3),
                    )
                nc.vector.tensor_add(h0[j][:], ps[:], encb[:])
                hb = workp.tile([128, 128], BF, name="hb0", tag="hb")
                nc.vector.tensor_copy(hb[:], h0[j][:])
                pst = psump.tile([128, 128], BF, name="trps0", tag="tr", bufs=2)
                nc.tensor.transpose(pst[:], hb[:], ident[:])
                nc.scalar.copy(hT0[:, 128 * j : 128 * (j + 1)], pst[:])

            # ---- one GCN layer ----
            def gcn_layer(lidx, h_in, hT_in, Wt, bt, nm, keep_bf=None):
                # support = h @ W (node-major, bf16), store p-major to DRAM
                supp = suppp.tile([128, JJ, HID], BF, name=f"supp{nm}", tag="supp")
                for j in range(NQ):
                    ps = psump.tile(
                        [128, HID], F32, name=f"sps{nm}", tag="mm512", bufs=3
                    )
                    nc.tensor.matmul(
                        ps[:],
                        hT_in[:, 128 * j : 128 * (j + 1)],
                        Wt[:],
                        start=True,
                        stop=True,
                    )
                    nc.vector.tensor_copy(supp[:, j, :], ps[:])
                cc_in = dramp.tile([NLOC, HID], BF, name=f"ccin{nm}", tag="ccin")
                nc.sync.dma_start(
                    cc_in[:].rearrange("(p j) f -> p j f", p=128), supp[:]
                )
                cc_out = dramp.tile(
                    [N, HID], BF, name=f"ccout{nm}", tag="ccout", addr_space="Shared"
                )
                nc.gpsimd.collective_compute(
                    "AllGather",
                    OP.bypass,
                    replica_groups=[list(range(NCORES))],
                    ins=[cc_in.opt()],
                    outs=[cc_out.opt()],
                )

                h_out = new_hgen(nm)
                hT_out = htp.tile([128, NLOC], BF, name=f"hT{nm}", tag="ht")
                for q in range(NQ):
                    nT = T_quad[q]
                    nidx = nT * 128
                    msgs = msgsp.tile(
                        [128, Tq_max, HID], BF, name=f"msgs{nm}", tag="msgs"
                    )
                    nc.gpsimd.dma_gather(
                        msgs[:, :nT, :],
                        cc_out[:],
                        idx_sb[:, TB[WPQ * q] * 8 : TB[WPQ * (q + 1)] * 8],
                        nidx,
                        gs_reg(nidx),
                        HID,
                        single_packet=False,
                    )
                    ps = psump.tile([128, HID], F32, name=f"segps{nm}", tag="seg", bufs=3)
                    for wi in range(WPQ):
                        wdx = WPQ * q + wi
                        tw = int(T_w[wdx])
                        base = int(TB[wdx] - TB[WPQ * q])
                        for t in range(tw):
                            nc.tensor.matmul(
                                ps[32 * wi : 32 * (wi + 1), :],
                                m_sb[:, int(TB[wdx]) + t, :],
                                msgs[:, base + t, :],
                                start=(t == 0),
                                stop=(t == tw - 1),
                                tile_position=(0, 32 * wi),
                            )
                    # finalize: out = beta*h + agg ; out /= max(||out||, eps); +b; relu
                    res = workp.tile([128, HID], F32, name=f"res{nm}", tag="res")
                    nc.vector.scalar_tensor_tensor(
                        res[:], h_in[q][:], float(BETA), ps[:], OP.mult, OP.add
                    )
                    sq = workp.tile([128, HID], F32, name=f"sq{nm}", tag="sq")
                    nsq = smallp.tile([128, 1], F32, name=f"nsq{nm}", tag="nsq")
                    nc.scalar.activation(sq[:], res[:], AF.Square, accum_out=nsq[:])
                    nrm = smallp.tile([128, 1], F32, name=f"nrm{nm}", tag="nrm")
                    nc.scalar.activation(nrm[:], nsq[:], AF.Sqrt)
                    nrm2 = smallp.tile([128, 1], F32, name=f"nrm2{nm}", tag="nrm2")
                    nc.vector.tensor_scalar_max(nrm2[:], nrm[:], float(EPS))
                    rnm = smallp.tile([128, 1], F32, name=f"rnm{nm}", tag="rnm")
                    nc.vector.reciprocal(rnm[:], nrm2[:])
                    scaled = workp.tile([128, HID], F32, name=f"scl{nm}", tag="scl")
                    nc.scalar.mul(scaled[:], res[:], rnm[:])
                    ho = h_out[q]
                    nc.vector.tensor_add(ho[:], scaled[:], bt[:])
                    nc.vector.tensor_scalar_max(ho[:], ho[:], 0.0)
                    if keep_bf is None:
                        hb = workp.tile([128, 128], BF, name=f"hb{nm}", tag="hb")
                    else:
                        hb = keep_bf[:, q, :]
                    nc.vector.tensor_copy(hb[:], ho[:])
                    pst = psump.tile(
                        [128, 128], BF, name=f"trps{nm}", tag="tr", bufs=2
                    )
                    nc.tensor.transpose(pst[:], hb[:], ident[:])
                    nc.scalar.copy(hT_out[:, 128 * q : 128 * (q + 1)], pst[:])
                return h_out, hT_out

            h1, hT1 = gcn_layer(0, h0, hT0, gcw[0], gcb[0], "e1")
            h2, hT2 = gcn_layer(1, h1, hT1, gcw[1], gcb[1], "e2")

            # structure decoder layer; keep bf16 copy of s for the AllGather
            s_bf = suppp.tile([128, JJ, HID], BF, name="s_bf", tag="sbf", bufs=1)
            _s, sT = gcn_layer(4, h2, hT2, gcw[4], gcb[4], "s", keep_bf=s_bf)
            s_cc_in = dramp.tile([NLOC, HID], BF, name="s_ccin", tag="ccin")
            nc.sync.dma_start(
                s_cc_in[:].rearrange("(p j) f -> p j f", p=128), s_bf[:]
            )
            s_cc_out = dramp.tile(
                [N, HID], BF, name="s_ccout", tag="ccout", addr_space="Shared"
            )
            nc.gpsimd.collective_compute(
                "AllGather",
                OP.bypass,
                replica_groups=[list(range(NCORES))],
                ins=[s_cc_in.opt()],
                outs=[s_cc_out.opt()],
            )
            sfT = bigp.tile([128, N], BF, name="sfT", tag="big")
            nc.sync.dma_start(sfT[:], s_cc_out[:], transpose=True)

            # attribute decoder branch
            a1, aT1 = gcn_layer(2, h2, hT2, gcw[2], gcb[2], "a1")
            a2, aT2 = gcn_layer(3, a1, aT1, gcw[3], gcb[3], "a2")

            # x_hat = a2 @ attrW + attrb
            for j in range(NQ):
                ps = psump.tile([128, FEAT], F32, name="xps", tag="mm512", bufs=3)
                nc.tensor.matmul(
                    ps[:],
                    aT2[:, 128 * j : 128 * (j + 1)],
                    attrw[:],
                    start=True,
                    stop=True,
                )
                xst = stagep.tile([128, FEAT], F32, name="xst", tag="xst")
                nc.vector.tensor_add(xst[:], ps[:], attrb[:])
                nc.sync.dma_start(xhat_out[128 * j : 128 * (j + 1), :], xst[:])

            # a_hat = s @ s_full.T  (columns unpermuted during PSUM->SBUF copy)
            cp_flip = 0
            for j in range(NQ):
                for cb in range(8):
                    st = stagep.tile([128, NLOC], F32, name="ast", tag="ast")
                    # natural cols n = 128*jj + 32*m + pr ; st viewed [pr, jj]
                    stv = st[:].rearrange("p (jj pp) -> p pp jj", jj=JJ)
                    for m in range(4):
                        ps = psump.tile(
                            [128, 512], F32, name="aps", tag="mm512", bufs=3
                        )
                        nc.tensor.matmul(
                            ps[:],
                            sT[:, 128 * j : 128 * (j + 1)],
                            sfT[:, 2048 * cb + 512 * m : 2048 * cb + 512 * (m + 1)],
                            start=True,
                            stop=True,
                        )
                        psv = ps[:].rearrange("p (pr jj) -> p pr jj", pr=32)
                        if cp_flip % 2 == 0:
                            nc.vector.tensor_copy(
                                stv[:, 32 * m : 32 * (m + 1), :], psv
                            )
                        else:
                            nc.scalar.copy(stv[:, 32 * m : 32 * (m + 1), :], psv)
                        cp_flip += 1
                    nc.sync.dma_start(
                        ahat_out[
                            128 * j : 128 * (j + 1), 2048 * cb : 2048 * (cb + 1)
                        ],
                        st[:],
                    )

    nc.compile()
    return nc


def prep_inputs(x, prep, core):
    """Per-core input map (numpy arrays matching declared dtypes)."""
    xl = np.asarray(x[core * NLOC : (core + 1) * NLOC], np.float32)  # [2048, 512]
    xt = np.ascontiguousarray(
        xl.T.reshape(4, 128, NLOC).transpose(1, 0, 2)
    ).astype(BF_NP)
    mt = np.ascontiguousarray(
        prep["M"][core].reshape(128, prep["T_total"] * RW)
    ).astype(BF_NP)
    idx = np.ascontiguousarray(prep["idx16"][core])
    return dict(xt=xt, mt=mt, idx=idx)


def shared_inputs(enc_lin_W, enc_lin_b, enc_gc1_W, enc_gc1_b, enc_gc2_W, enc_gc2_b,
                  attr_gc1_W, attr_gc1_b, attr_gc2_W, attr_gc2_b,
                  attr_lin_W, attr_lin_b, str_gc1_W, str_gc1_b):
    def bcast(b, width):
        return np.ascontiguousarray(
            np.broadcast_to(np.asarray(b, np.float32), (128, width))
        )

    d = dict(
        encw=np.ascontiguousarray(
            np.asarray(enc_lin_W, np.float32).reshape(4, 128, HID).transpose(1, 0, 2)
        ).astype(BF_NP),
        encb=bcast(enc_lin_b, HID),
        attrw=np.asarray(attr_lin_W, np.float32).astype(BF_NP),
        attrb=bcast(attr_lin_b, FEAT),
        ident=np.eye(128, dtype=BF_NP),
    )
    gws = [enc_gc1_W, enc_gc2_W, attr_gc1_W, attr_gc2_W, str_gc1_W]
    gbs = [enc_gc1_b, enc_gc2_b, attr_gc1_b, attr_gc2_b, str_gc1_b]
    for l in range(5):
        d[f"gcw{l}"] = np.asarray(gws[l], np.float32).astype(BF_NP)
        d[f"gcb{l}"] = bcast(gbs[l], HID)
    return d


_BUILD_CACHE = {}


def kernel(x, edge_index, edge_weight,
           enc_lin_W, enc_lin_b, enc_gc1_W, enc_gc1_b, enc_gc2_W, enc_gc2_b,
           attr_gc1_W, attr_gc1_b, attr_gc2_W, attr_gc2_b, attr_lin_W, attr_lin_b,
           str_gc1_W, str_gc1_b, _trace=False):
    x = np.asarray(x, np.float32)
    edge_index = np.asarray(edge_index)
    edge_weight = np.asarray(edge_weight, np.float32)

    prep = prep_graph(edge_index, edge_weight)
    ck = prep["M"].tobytes()[:0]  # program depends only on T_w/TB
    key = (tuple(prep["T_w"].tolist()),)
    if key not in _BUILD_CACHE:
        _BUILD_CACHE[key] = build_program(prep["T_w"], prep["TB"], prep["T_total"])
    nc = _BUILD_CACHE[key]

    shared = shared_inputs(
        enc_lin_W, enc_lin_b, enc_gc1_W, enc_gc1_b, enc_gc2_W, enc_gc2_b,
        attr_gc1_W, attr_gc1_b, attr_gc2_W, attr_gc2_b, attr_lin_W, attr_lin_b,
        str_gc1_W, str_gc1_b,
    )
    in_maps = []
    for c in range(NCORES):
        m = prep_inputs(x, prep, c)
        m.update(shared)
        in_maps.append(m)

    res = run_bass_kernel_spmd(nc, in_maps, list(range(NCORES)), trace=_trace)

    a_hat = np.concatenate([res.results[c]["ahat"] for c in range(NCORES)], axis=0)
    x_hat = np.concatenate([res.results[c]["xhat"] for c in range(NCORES)], axis=0)
    if _trace:
        kernel.last_exec_time_ns = res.exec_time_ns
        kernel.last_results = res
    return (a_hat, x_hat)


# revision 6
# speedup vs baseline: 1.3344x; 1.3344x over previous
"""Trainium2 Bass kernel for nn_DominantNeigh (GCN autoencoder, DOMINANT-style).

Distribution: nodes row-sharded across 8 NeuronCores (2048 rows/core); edges
partitioned by destination row. Per GCN layer each core computes its support
shard (h @ W in bf16 on the PE), all-gathers support into a DRAM buffer
(p-major row permutation so the SBUF->DRAM store is one contiguous DMA), then
edge messages are fetched with dma_gather (128 edges land on 128 partitions)
and the weighted segment-sum is a PE matmul against host-precomputed one-hot
x weight scatter matrices, accumulated in PSUM in 32-row dst windows (4
windows per PSUM tile via col-group tile_position). Residual + L2 row-norm +
bias + relu run on DVE/ACT from PSUM. a_hat = s @ s.T is row-sharded
([2048,16384] f32 per core, the dominant HBM write); x_hat via attr decoder.
"""

import sys

sys.path.insert(0, "/opt/trn_rl_repo")

import numpy as np
import ml_dtypes

from concourse import bacc, bass, tile
from concourse import mybir
from concourse.bass_utils import run_bass_kernel_spmd

# problem shape (hardcoded; kernel.py must be self-contained)
N, FEAT, HID, E = 16384, 512, 128, 524288
BETA, EPS = 0.001, 1e-12
NCORES = 8
NLOC = N // NCORES  # 2048
RW = 32  # dst rows per window (one 32-col PE group)
NWIN = NLOC // RW  # 64 windows per core
WPQ = 4  # windows per PSUM quad tile (128 rows)
NQ = NWIN // WPQ  # 16 quads per core
JJ = NLOC // 128  # 16 sub-rows per partition in p-major shard layout

BF = mybir.dt.bfloat16
F32 = mybir.dt.float32
I16 = mybir.dt.int16
BF_NP = ml_dtypes.bfloat16
AF = mybir.ActivationFunctionType
OP = mybir.AluOpType


def prep_graph(edge_index, edge_weight):
    """Host-side: dedup (dst,src) edges, partition by dst core/window, sort by
    gather address, build per-core scatter matrices M and gather indices.

    Returns dict with T_w [NWIN], TB [NWIN+1], T_total, M [NCORES,128,Tt,RW]
    f32, idx16 [NCORES,128,Tt*8] int16 (replicated over partition groups).
    """
    dst = np.asarray(edge_index[0], dtype=np.int64)
    src = np.asarray(edge_index[1], dtype=np.int64)
    w = np.asarray(edge_weight, dtype=np.float64) * (1.0 - BETA)

    # dedup (dst, src): segment_sum adds duplicate edges
    key = dst * N + src
    order = np.argsort(key, kind="stable")
    ks, ws = key[order], w[order]
    newgrp = np.ones(len(ks), bool)
    newgrp[1:] = ks[1:] != ks[:-1]
    gid = np.cumsum(newgrp) - 1
    wsum = np.zeros(int(gid[-1]) + 1)
    np.add.at(wsum, gid, ws)
    ukey = ks[newgrp]
    udst, usrc = ukey // N, ukey % N
    # p-major permuted row of src in the all-gathered buffer:
    # shard row r = (n%128)*JJ + n//128 for local n
    sloc = usrc % NLOC
    uv = (usrc // NLOC) * NLOC + (sloc % 128) * JJ + sloc // 128
    core = udst // NLOC
    dloc = udst % NLOC
    winloc = dloc // RW
    roff = dloc % RW

    # order by (core, window, gather address) - address order gives HBM locality
    o2 = np.lexsort((uv, winloc, core))
    core, winloc, roff, uv, wsum = core[o2], winloc[o2], roff[o2], uv[o2], wsum[o2]

    counts = np.zeros((NCORES, NWIN), np.int64)
    np.add.at(counts, (core, winloc), 1)
    T_w = np.maximum(1, -(-counts.max(axis=0) // 128))  # tiles per window
    TB = np.zeros(NWIN + 1, np.int64)
    TB[1:] = np.cumsum(T_w)
    T_total = int(TB[-1])

    # position of each entry within its (core, window) group
    gkey = core * NWIN + winloc
    newg = np.ones(len(gkey), bool)
    newg[1:] = gkey[1:] != gkey[:-1]
    gidx = np.cumsum(newg) - 1
    starts = np.flatnonzero(newg)
    pos = np.arange(len(gkey)) - starts[gidx]

    q = winloc // WPQ
    slot = (TB[winloc] - TB[WPQ * q]) * 128 + pos  # call-local gather slot
    tile_g = TB[winloc] + pos // 128
    part = pos % 128

    M = np.zeros((NCORES, 128, T_total, RW), np.float32)
    M[core, part, tile_g, roff] = wsum

    idx16 = np.zeros((NCORES, 16, T_total * 8), np.int16)
    colv = TB[WPQ * q] * 8 + slot // 16
    idx16[core, slot % 16, colv] = uv.astype(np.int16)
    idx16 = np.tile(idx16, (1, 8, 1))  # replicate to 128 partitions

    return dict(T_w=T_w, TB=TB, T_total=T_total, M=M, idx16=idx16)


def build_program(T_w, TB, T_total):
    """Build the SPMD Bass program (shared by all 8 cores)."""
    T_quad = [int(TB[WPQ * (q + 1)] - TB[WPQ * q]) for q in range(NQ)]
    Tq_max = max(T_quad)

    nc = bacc.Bacc(num_devices=NCORES, num_swdge_queues=4)

    xt_in = nc.dram_tensor("xt", [128, 4, NLOC], BF, kind="ExternalInput")
    m_in = nc.dram_tensor("mt", [128, T_total * RW], BF, kind="ExternalInput")
    idx_in = nc.dram_tensor("idx", [128, T_total * 8], I16, kind="ExternalInput")
    encw_in = nc.dram_tensor("encw", [128, 4, HID], BF, kind="ExternalInput")
    encb_in = nc.dram_tensor("encb", [128, HID], F32, kind="ExternalInput")
    gcw_in = [
        nc.dram_tensor(f"gcw{l}", [HID, HID], BF, kind="ExternalInput")
        for l in range(5)
    ]
    gcb_in = [
        nc.dram_tensor(f"gcb{l}", [128, HID], F32, kind="ExternalInput")
        for l in range(5)
    ]
    attrw_in = nc.dram_tensor("attrw", [HID, FEAT], BF, kind="ExternalInput")
    attrb_in = nc.dram_tensor("attrb", [128, FEAT], F32, kind="ExternalInput")
    ident_in = nc.dram_tensor("ident", [128, 128], BF, kind="ExternalInput")
    ahat_out = nc.dram_tensor("ahat", [NLOC, N], F32, kind="ExternalOutput")
    xhat_out = nc.dram_tensor("xhat", [NLOC, FEAT], F32, kind="ExternalOutput")

    with tile.TileContext(nc) as tc:
        with (
            tc.tile_pool(name="const", bufs=1) as constp,
            tc.tile_pool(name="big", bufs=1) as bigp,
            tc.tile_pool(name="hp", bufs=48) as hp,
            tc.tile_pool(name="htp", bufs=4) as htp,
            tc.tile_pool(name="msgsp", bufs=3) as msgsp,
            tc.tile_pool(name="workp", bufs=4) as workp,
            tc.tile_pool(name="smallp", bufs=8) as smallp,
            tc.tile_pool(name="suppp", bufs=2) as suppp,
            tc.tile_pool(name="stagep", bufs=2) as stagep,
            tc.tile_pool(name="psum", bufs=1, space="PSUM") as psump,
            tc.tile_pool(name="dramp", bufs=2, space="DRAM") as dramp,
        ):
            # ---- constants ----
            ident = constp.tile([128, 128], BF)
            nc.sync.dma_start(ident[:], ident_in[:])
            encw = constp.tile([128, 4, HID], BF)
            nc.sync.dma_start(encw[:], encw_in[:])
            encb = constp.tile([128, HID], F32)
            nc.sync.dma_start(encb[:], encb_in[:])
            gcw, gcb = [], []
            for l in range(5):
                wt = constp.tile([HID, HID], BF, name=f"gcw{l}")
                nc.sync.dma_start(wt[:], gcw_in[l][:])
                gcw.append(wt)
                bt = constp.tile([128, HID], F32, name=f"gcb{l}")
                nc.sync.dma_start(bt[:], gcb_in[l][:])
                gcb.append(bt)
            attrw = constp.tile([HID, FEAT], BF)
            nc.sync.dma_start(attrw[:], attrw_in[:])
            attrb = constp.tile([128, FEAT], F32)
            nc.sync.dma_start(attrb[:], attrb_in[:])
            m_sb = constp.tile([128, T_total, RW], BF)
            nc.sync.dma_start(m_sb[:], m_in[:].rearrange("p (t r) -> p t r", r=RW))
            idx_sb = constp.tile([128, T_total * 8], I16)
            nc.sync.dma_start(idx_sb[:], idx_in[:])
            xt = bigp.tile([128, 4, NLOC], BF, tag="big")
            nc.sync.dma_start(xt[:], xt_in[:])

            # cached scalar registers for dma_gather num_idxs
            reg_cache = {}

            def gs_reg(v):
                if v not in reg_cache:
                    reg_cache[v] = nc.gpsimd.to_reg(v)
                return reg_cache[v]

            def new_hgen(nm):
                return [
                    hp.tile([128, 128], F32, name=f"h_{nm}_{j}", tag="h")
                    for j in range(NQ)
                ]

            # ---- encoder linear: h0 = x @ encW + encb (node-major) ----
            h0 = new_hgen("h0")
            hT0 = htp.tile([128, NLOC], BF, name="hT0", tag="ht")
            for j in range(NQ):
                ps = psump.tile([128, HID], F32, name="encps", tag="mm512", bufs=3)
                for k in range(4):
                    nc.tensor.matmul(
                        ps[:],
                        xt[:, k, 128 * j : 128 * (j + 1)],
                        encw[:, k, :],
                        start=(k == 0),
                        stop=(k == 3),
                    )
                nc.vector.tensor_add(h0[j][:], ps[:], encb[:])
                hb = workp.tile([128, 128], BF, name="hb0", tag="hb")
                nc.scalar.copy(hb[:], h0[j][:])
                pst = psump.tile([128, 128], BF, name="trps0", tag="tr", bufs=2)
                nc.tensor.transpose(pst[:], hb[:], ident[:])
                nc.scalar.copy(hT0[:, 128 * j : 128 * (j + 1)], pst[:])

            # ---- one GCN layer ----
            def gcn_layer(lidx, h_in, hT_in, Wt, bt, nm, keep_bf=None):
                # support = h @ W (node-major, bf16), store p-major to DRAM
                supp = suppp.tile([128, JJ, HID], BF, name=f"supp{nm}", tag="supp")
                for j in range(NQ):
                    ps = psump.tile(
                        [128, HID], F32, name=f"sps{nm}", tag="mm512", bufs=3
                    )
                    nc.tensor.matmul(
                        ps[:],
                        hT_in[:, 128 * j : 128 * (j + 1)],
                        Wt[:],
                        start=True,
                        stop=True,
                    )
                    nc.vector.tensor_copy(supp[:, j, :], ps[:])
                cc_in = dramp.tile([NLOC, HID], BF, name=f"ccin{nm}", tag="ccin")
                nc.sync.dma_start(
                    cc_in[:].rearrange("(p j) f -> p j f", p=128), supp[:]
                )
                cc_out = dramp.tile(
                    [N, HID], BF, name=f"ccout{nm}", tag="ccout", addr_space="Shared"
                )
                nc.gpsimd.collective_compute(
                    "AllGather",
                    OP.bypass,
                    replica_groups=[list(range(NCORES))],
                    ins=[cc_in.opt()],
                    outs=[cc_out.opt()],
                )

                h_out = new_hgen(nm)
                hT_out = htp.tile([128, NLOC], BF, name=f"hT{nm}", tag="ht")
                for q in range(NQ):
                    nT = T_quad[q]
                    nidx = nT * 128
                    msgs = msgsp.tile(
                        [128, Tq_max, HID], BF, name=f"msgs{nm}", tag="msgs"
                    )
                    nc.gpsimd.dma_gather(
                        msgs[:, :nT, :],
                        cc_out[:],
                        idx_sb[:, TB[WPQ * q] * 8 : TB[WPQ * (q + 1)] * 8],
                        nidx,
                        gs_reg(nidx),
                        HID,
                        single_packet=False,
                        queue_num=q % 4,
                    )
                    ps = psump.tile([128, HID], F32, name=f"segps{nm}", tag="seg", bufs=3)
                    for wi in range(WPQ):
                        wdx = WPQ * q + wi
                        tw = int(T_w[wdx])
                        base = int(TB[wdx] - TB[WPQ * q])
                        for t in range(tw):
                            nc.tensor.matmul(
                                ps[32 * wi : 32 * (wi + 1), :],
                                m_sb[:, int(TB[wdx]) + t, :],
                                msgs[:, base + t, :],
                                start=(t == 0),
                                stop=(t == tw - 1),
                                tile_position=(0, 32 * wi),
                            )
                    # finalize: out = beta*h + agg ; out /= max(||out||, eps); +b; relu
                    res = workp.tile([128, HID], F32, name=f"res{nm}", tag="res")
                    nc.vector.scalar_tensor_tensor(
                        res[:], h_in[q][:], float(BETA), ps[:], OP.mult, OP.add
                    )
                    sq = workp.tile([128, HID], F32, name=f"sq{nm}", tag="sq")
                    nsq = smallp.tile([128, 1], F32, name=f"nsq{nm}", tag="nsq")
                    nc.scalar.activation(sq[:], res[:], AF.Square, accum_out=nsq[:])
                    nrm = smallp.tile([128, 1], F32, name=f"nrm{nm}", tag="nrm")
                    nc.scalar.activation(nrm[:], nsq[:], AF.Sqrt)
                    nrm2 = smallp.tile([128, 1], F32, name=f"nrm2{nm}", tag="nrm2")
                    nc.vector.scalar_tensor_tensor(
                        nrm2[:], nrm[:], float(EPS), nrm[:], OP.max, OP.max
                    )
                    rnm = smallp.tile([128, 1], F32, name=f"rnm{nm}", tag="rnm")
                    nc.vector.reciprocal(rnm[:], nrm2[:])
                    scaled = workp.tile([128, HID], F32, name=f"scl{nm}", tag="scl")
                    nc.scalar.mul(scaled[:], res[:], rnm[:])
                    hosum = workp.tile([128, HID], F32, name=f"hos{nm}", tag="hos")
                    nc.vector.scalar_tensor_tensor(
                        hosum[:], scaled[:], 1.0, bt[:], OP.mult, OP.add
                    )
                    ho = h_out[q]
                    nc.scalar.activation(ho[:], hosum[:], AF.Relu)
                    if keep_bf is None:
                        hb = workp.tile([128, 128], BF, name=f"hb{nm}", tag="hb")
                    else:
                        hb = keep_bf[:, q, :]
                    nc.scalar.copy(hb[:], ho[:])
                    pst = psump.tile(
                        [128, 128], BF, name=f"trps{nm}", tag="tr", bufs=2
                    )
                    nc.tensor.transpose(pst[:], hb[:], ident[:])
                    nc.scalar.copy(hT_out[:, 128 * q : 128 * (q + 1)], pst[:])
                return h_out, hT_out

            h1, hT1 = gcn_layer(0, h0, hT0, gcw[0], gcb[0], "e1")
            h2, hT2 = gcn_layer(1, h1, hT1, gcw[1], gcb[1], "e2")

            # structure decoder layer; keep bf16 copy of s for the AllGather
            s_bf = suppp.tile([128, JJ, HID], BF, name="s_bf", tag="sbf", bufs=1)
            _s, sT = gcn_layer(4, h2, hT2, gcw[4], gcb[4], "s", keep_bf=s_bf)
            s_cc_in = dramp.tile([NLOC, HID], BF, name="s_ccin", tag="ccin")
            nc.sync.dma_start(
                s_cc_in[:].rearrange("(p j) f -> p j f", p=128), s_bf[:]
            )
            s_cc_out = dramp.tile(
                [N, HID], BF, name="s_ccout", tag="ccout", addr_space="Shared"
            )
            nc.gpsimd.collective_compute(
                "AllGather",
                OP.bypass,
                replica_groups=[list(range(NCORES))],
                ins=[s_cc_in.opt()],
                outs=[s_cc_out.opt()],
            )
            sfT = bigp.tile([128, N], BF, name="sfT", tag="big")
            nc.sync.dma_start(sfT[:], s_cc_out[:], transpose=True)

            # attribute decoder branch
            a1, aT1 = gcn_layer(2, h2, hT2, gcw[2], gcb[2], "a1")
            a2, aT2 = gcn_layer(3, a1, aT1, gcw[3], gcb[3], "a2")

            # x_hat = a2 @ attrW + attrb
            for j in range(NQ):
                ps = psump.tile([128, FEAT], F32, name="xps", tag="mm512", bufs=3)
                nc.tensor.matmul(
                    ps[:],
                    aT2[:, 128 * j : 128 * (j + 1)],
                    attrw[:],
                    start=True,
                    stop=True,
                )
                xst = stagep.tile([128, FEAT], F32, name="xst", tag="xst")
                nc.vector.tensor_add(xst[:], ps[:], attrb[:])
                nc.sync.dma_start(xhat_out[128 * j : 128 * (j + 1), :], xst[:])

            # a_hat = s @ s_full.T  (columns unpermuted during PSUM->SBUF copy)
            cp_flip = 0
            for j in range(NQ):
                for cb in range(8):
                    st = stagep.tile([128, NLOC], F32, name="ast", tag="ast")
                    # natural cols n = 128*jj + 32*m + pr ; st viewed [pr, jj]
                    stv = st[:].rearrange("p (jj pp) -> p pp jj", jj=JJ)
                    for m in range(4):
                        ps = psump.tile(
                            [128, 512], F32, name="aps", tag="mm512", bufs=3
                        )
                        nc.tensor.matmul(
                            ps[:],
                            sT[:, 128 * j : 128 * (j + 1)],
                            sfT[:, 2048 * cb + 512 * m : 2048 * cb + 512 * (m + 1)],
                            start=True,
                            stop=True,
                        )
                        psv = ps[:].rearrange("p (pr jj) -> p pr jj", pr=32)
                        if cp_flip % 2 == 0:
                            nc.vector.tensor_copy(
                                stv[:, 32 * m : 32 * (m + 1), :], psv
                            )
                        else:
                            nc.scalar.copy(stv[:, 32 * m : 32 * (m + 1), :], psv)
                        cp_flip += 1
                    nc.sync.dma_start(
                        ahat_out[
                            128 * j : 128 * (j + 1), 2048 * cb : 2048 * (cb + 1)
                        ],
                        st[:],
                    )

    nc.compile()
    return nc


def prep_inputs(x, prep, core):
    """Per-core input map (numpy arrays matching declared dtypes)."""
    xl = np.asarray(x[core * NLOC : (core + 1) * NLOC], np.float32)  # [2048, 512]
    xt = np.ascontiguousarray(
        xl.T.reshape(4, 128, NLOC).transpose(1, 0, 2)
    ).astype(BF_NP)
    mt = np.ascontiguousarray(
        prep["M"][core].reshape(128, prep["T_total"] * RW)
    ).astype(BF_NP)
    idx = np.ascontiguousarray(prep["idx16"][core])
    return dict(xt=xt, mt=mt, idx=idx)


def shared_inputs(enc_lin_W, enc_lin_b, enc_gc1_W, enc_gc1_b, enc_gc2_W, enc_gc2_b,
                  attr_gc1_W, attr_gc1_b, attr_gc2_W, attr_gc2_b,
                  attr_lin_W, attr_lin_b, str_gc1_W, str_gc1_b):
    def bcast(b, width):
        return np.ascontiguousarray(
            np.broadcast_to(np.asarray(b, np.float32), (128, width))
        )

    d = dict(
        encw=np.ascontiguousarray(
            np.asarray(enc_lin_W, np.float32).reshape(4, 128, HID).transpose(1, 0, 2)
        ).astype(BF_NP),
        encb=bcast(enc_lin_b, HID),
        attrw=np.asarray(attr_lin_W, np.float32).astype(BF_NP),
        attrb=bcast(attr_lin_b, FEAT),
        ident=np.eye(128, dtype=BF_NP),
    )
    gws = [enc_gc1_W, enc_gc2_W, attr_gc1_W, attr_gc2_W, str_gc1_W]
    gbs = [enc_gc1_b, enc_gc2_b, attr_gc1_b, attr_gc2_b, str_gc1_b]
    for l in range(5):
        d[f"gcw{l}"] = np.asarray(gws[l], np.float32).astype(BF_NP)
        d[f"gcb{l}"] = bcast(gbs[l], HID)
    return d


_BUILD_CACHE = {}


def kernel(x, edge_index, edge_weight,
           enc_lin_W, enc_lin_b, enc_gc1_W, enc_gc1_b, enc_gc2_W, enc_gc2_b,
           attr_gc1_W, attr_gc1_b, attr_gc2_W, attr_gc2_b, attr_lin_W, attr_lin_b,
           str_gc1_W, str_gc1_b, _trace=False):
    x = np.asarray(x, np.float32)
    edge_index = np.asarray(edge_index)
    edge_weight = np.asarray(edge_weight, np.float32)

    prep = prep_graph(edge_index, edge_weight)
    ck = prep["M"].tobytes()[:0]  # program depends only on T_w/TB
    key = (tuple(prep["T_w"].tolist()),)
    if key not in _BUILD_CACHE:
        _BUILD_CACHE[key] = build_program(prep["T_w"], prep["TB"], prep["T_total"])
    nc = _BUILD_CACHE[key]

    shared = shared_inputs(
        enc_lin_W, enc_lin_b, enc_gc1_W, enc_gc1_b, enc_gc2_W, enc_gc2_b,
        attr_gc1_W, attr_gc1_b, attr_gc2_W, attr_gc2_b, attr_lin_W, attr_lin_b,
        str_gc1_W, str_gc1_b,
    )
    in_maps = []
    for c in range(NCORES):
        m = prep_inputs(x, prep, c)
        m.update(shared)
        in_maps.append(m)

    res = run_bass_kernel_spmd(nc, in_maps, list(range(NCORES)), trace=_trace)

    a_hat = np.concatenate([res.results[c]["ahat"] for c in range(NCORES)], axis=0)
    x_hat = np.concatenate([res.results[c]["xhat"] for c in range(NCORES)], axis=0)
    if _trace:
        kernel.last_exec_time_ns = res.exec_time_ns
        kernel.last_results = res
    return (a_hat, x_hat)


# revision 7
# speedup vs baseline: 1.5158x; 1.1360x over previous
"""Trainium2 Bass kernel for nn_DominantNeigh (GCN autoencoder, DOMINANT-style).

Distribution: nodes row-sharded across 8 NeuronCores (2048 rows/core); edges
partitioned by destination row. Per GCN layer each core computes its support
shard (h @ W in bf16 on the PE), all-gathers support into a DRAM buffer
(p-major row permutation so the SBUF->DRAM store is one contiguous DMA), then
edge messages are fetched with dma_gather (128 edges land on 128 partitions)
and the weighted segment-sum is a PE matmul against host-precomputed one-hot
x weight scatter matrices, accumulated in PSUM in 32-row dst windows (4
windows per PSUM tile via col-group tile_position). Residual + L2 row-norm +
bias + relu run on DVE/ACT from PSUM. a_hat = s @ s.T is row-sharded
([2048,16384] f32 per core, the dominant HBM write); x_hat via attr decoder.
"""

import sys

sys.path.insert(0, "/opt/trn_rl_repo")

import numpy as np
import ml_dtypes

from concourse import bacc, bass, tile
from concourse import mybir
from concourse.bass_utils import run_bass_kernel_spmd

# problem shape (hardcoded; kernel.py must be self-contained)
N, FEAT, HID, E = 16384, 512, 128, 524288
BETA, EPS = 0.001, 1e-12
NCORES = 8
NLOC = N // NCORES  # 2048
RW = 32  # dst rows per window (one 32-col PE group)
NWIN = NLOC // RW  # 64 windows per core
WPQ = 4  # windows per PSUM quad tile (128 rows)
NQ = NWIN // WPQ  # 16 quads per core
JJ = NLOC // 128  # 16 sub-rows per partition in p-major shard layout

BF = mybir.dt.bfloat16
F32 = mybir.dt.float32
I16 = mybir.dt.int16
BF_NP = ml_dtypes.bfloat16
AF = mybir.ActivationFunctionType
OP = mybir.AluOpType


def prep_graph(edge_index, edge_weight):
    """Host-side: dedup (dst,src) edges, partition by dst core/window, sort by
    gather address, build per-core scatter matrices M and gather indices.

    Returns dict with T_w [NWIN], TB [NWIN+1], T_total, M [NCORES,128,Tt,RW]
    f32, idx16 [NCORES,128,Tt*8] int16 (replicated over partition groups).
    """
    dst = np.asarray(edge_index[0], dtype=np.int64)
    src = np.asarray(edge_index[1], dtype=np.int64)
    w = np.asarray(edge_weight, dtype=np.float64) * (1.0 - BETA)

    # dedup (dst, src): segment_sum adds duplicate edges
    key = dst * N + src
    order = np.argsort(key, kind="stable")
    ks, ws = key[order], w[order]
    newgrp = np.ones(len(ks), bool)
    newgrp[1:] = ks[1:] != ks[:-1]
    gid = np.cumsum(newgrp) - 1
    wsum = np.zeros(int(gid[-1]) + 1)
    np.add.at(wsum, gid, ws)
    ukey = ks[newgrp]
    udst, usrc = ukey // N, ukey % N
    # p-major permuted row of src in the all-gathered buffer:
    # shard row r = (n%128)*JJ + n//128 for local n
    sloc = usrc % NLOC
    uv = (usrc // NLOC) * NLOC + (sloc % 128) * JJ + sloc // 128
    core = udst // NLOC
    dloc = udst % NLOC
    winloc = dloc // RW
    roff = dloc % RW

    # order by (core, window, gather address) - address order gives HBM locality
    o2 = np.lexsort((uv, winloc, core))
    core, winloc, roff, uv, wsum = core[o2], winloc[o2], roff[o2], uv[o2], wsum[o2]

    counts = np.zeros((NCORES, NWIN), np.int64)
    np.add.at(counts, (core, winloc), 1)
    T_w = np.maximum(1, -(-counts.max(axis=0) // 128))  # tiles per window
    TB = np.zeros(NWIN + 1, np.int64)
    TB[1:] = np.cumsum(T_w)
    T_total = int(TB[-1])

    # position of each entry within its (core, window) group
    gkey = core * NWIN + winloc
    newg = np.ones(len(gkey), bool)
    newg[1:] = gkey[1:] != gkey[:-1]
    gidx = np.cumsum(newg) - 1
    starts = np.flatnonzero(newg)
    pos = np.arange(len(gkey)) - starts[gidx]

    q = winloc // WPQ
    slot = (TB[winloc] - TB[WPQ * q]) * 128 + pos  # call-local gather slot
    tile_g = TB[winloc] + pos // 128
    part = pos % 128

    M = np.zeros((NCORES, 128, T_total, RW), np.float32)
    M[core, part, tile_g, roff] = wsum

    idx16 = np.zeros((NCORES, 16, T_total * 8), np.int16)
    colv = TB[WPQ * q] * 8 + slot // 16
    idx16[core, slot % 16, colv] = uv.astype(np.int16)
    idx16 = np.tile(idx16, (1, 8, 1))  # replicate to 128 partitions

    return dict(T_w=T_w, TB=TB, T_total=T_total, M=M, idx16=idx16)


def build_program(T_w, TB, T_total):
    """Build the SPMD Bass program (shared by all 8 cores)."""
    T_quad = [int(TB[WPQ * (q + 1)] - TB[WPQ * q]) for q in range(NQ)]
    Tq_max = max(T_quad)

    nc = bacc.Bacc(num_devices=NCORES, num_swdge_queues=4)

    xt_in = nc.dram_tensor("xt", [128, 4, NLOC], BF, kind="ExternalInput")
    m_in = nc.dram_tensor("mt", [128, T_total * RW], BF, kind="ExternalInput")
    idx_in = nc.dram_tensor("idx", [128, T_total * 8], I16, kind="ExternalInput")
    encw_in = nc.dram_tensor("encw", [128, 4, HID], BF, kind="ExternalInput")
    encb_in = nc.dram_tensor("encb", [128, HID], F32, kind="ExternalInput")
    gcw_in = [
        nc.dram_tensor(f"gcw{l}", [HID, HID], BF, kind="ExternalInput")
        for l in range(5)
    ]
    gcb_in = [
        nc.dram_tensor(f"gcb{l}", [128, HID], F32, kind="ExternalInput")
        for l in range(5)
    ]
    attrw_in = nc.dram_tensor("attrw", [HID, FEAT], BF, kind="ExternalInput")
    attrb_in = nc.dram_tensor("attrb", [128, FEAT], F32, kind="ExternalInput")
    ident_in = nc.dram_tensor("ident", [128, 128], BF, kind="ExternalInput")
    ahat_out = nc.dram_tensor("ahat", [NLOC, N], F32, kind="ExternalOutput")
    xhat_out = nc.dram_tensor("xhat", [NLOC, FEAT], F32, kind="ExternalOutput")

    with tile.TileContext(nc) as tc:
        with (
            tc.tile_pool(name="const", bufs=1) as constp,
            tc.tile_pool(name="big", bufs=1) as bigp,
            tc.tile_pool(name="hp", bufs=44) as hp,
            tc.tile_pool(name="htp", bufs=4) as htp,
            tc.tile_pool(name="msgsp", bufs=6) as msgsp,
            tc.tile_pool(name="workp", bufs=3) as workp,
            tc.tile_pool(name="smallp", bufs=8) as smallp,
            tc.tile_pool(name="suppp", bufs=2) as suppp,
            tc.tile_pool(name="stagep", bufs=2) as stagep,
            tc.tile_pool(name="psum", bufs=1, space="PSUM") as psump,
            tc.tile_pool(name="dramp", bufs=2, space="DRAM") as dramp,
        ):
            # ---- constants ----
            ident = constp.tile([128, 128], BF)
            nc.sync.dma_start(ident[:], ident_in[:])
            encw = constp.tile([128, 4, HID], BF)
            nc.sync.dma_start(encw[:], encw_in[:])
            encb = constp.tile([128, HID], F32)
            nc.sync.dma_start(encb[:], encb_in[:])
            gcw, gcb = [], []
            for l in range(5):
                wt = constp.tile([HID, HID], BF, name=f"gcw{l}")
                nc.sync.dma_start(wt[:], gcw_in[l][:])
                gcw.append(wt)
                bt = constp.tile([128, HID], F32, name=f"gcb{l}")
                nc.sync.dma_start(bt[:], gcb_in[l][:])
                gcb.append(bt)
            attrw = constp.tile([HID, FEAT], BF)
            nc.sync.dma_start(attrw[:], attrw_in[:])
            attrb = constp.tile([128, FEAT], F32)
            nc.sync.dma_start(attrb[:], attrb_in[:])
            m_sb = constp.tile([128, T_total, RW], BF)
            nc.sync.dma_start(m_sb[:], m_in[:].rearrange("p (t r) -> p t r", r=RW))
            idx_sb = constp.tile([128, T_total * 8], I16)
            nc.sync.dma_start(idx_sb[:], idx_in[:])
            xt = bigp.tile([128, 4, NLOC], BF, tag="big")
            nc.sync.dma_start(xt[:], xt_in[:])

            # cached scalar registers for dma_gather num_idxs
            reg_cache = {}

            def gs_reg(v):
                if v not in reg_cache:
                    reg_cache[v] = nc.gpsimd.to_reg(v)
                return reg_cache[v]

            def new_hgen(nm):
                return [
                    hp.tile([128, 128], F32, name=f"h_{nm}_{j}", tag="h")
                    for j in range(NQ)
                ]

            # ---- encoder linear: h0 = x @ encW + encb (node-major) ----
            h0 = new_hgen("h0")
            hT0 = htp.tile([128, NLOC], BF, name="hT0", tag="ht")
            for j in range(NQ):
                ps = psump.tile([128, HID], F32, name="encps", tag="mm512", bufs=3)
                for k in range(4):
                    nc.tensor.matmul(
                        ps[:],
                        xt[:, k, 128 * j : 128 * (j + 1)],
                        encw[:, k, :],
                        start=(k == 0),
                        stop=(k == 3),
                    )
                nc.vector.tensor_add(h0[j][:], ps[:], encb[:])
                hb = workp.tile([128, 128], BF, name="hb0", tag="hb")
                nc.scalar.copy(hb[:], h0[j][:])
                pst = psump.tile([128, 128], BF, name="trps0", tag="tr", bufs=2)
                nc.tensor.transpose(pst[:], hb[:], ident[:])
                nc.scalar.copy(hT0[:, 128 * j : 128 * (j + 1)], pst[:])

            # ---- one GCN layer ----
            def gcn_layer(lidx, h_in, hT_in, Wt, bt, nm, keep_bf=None):
                # support = h @ W (node-major, bf16), store p-major to DRAM
                supp = suppp.tile([128, JJ, HID], BF, name=f"supp{nm}", tag="supp")
                for j in range(NQ):
                    ps = psump.tile(
                        [128, HID], F32, name=f"sps{nm}", tag="mm512", bufs=3
                    )
                    nc.tensor.matmul(
                        ps[:],
                        hT_in[:, 128 * j : 128 * (j + 1)],
                        Wt[:],
                        start=True,
                        stop=True,
                    )
                    nc.vector.tensor_copy(supp[:, j, :], ps[:])
                cc_in = dramp.tile([NLOC, HID], BF, name=f"ccin{nm}", tag="ccin")
                nc.sync.dma_start(
                    cc_in[:].rearrange("(p j) f -> p j f", p=128), supp[:]
                )
                cc_out = dramp.tile(
                    [N, HID], BF, name=f"ccout{nm}", tag="ccout", addr_space="Shared"
                )
                nc.gpsimd.collective_compute(
                    "AllGather",
                    OP.bypass,
                    replica_groups=[list(range(NCORES))],
                    ins=[cc_in.opt()],
                    outs=[cc_out.opt()],
                )

                h_out = new_hgen(nm)
                hT_out = htp.tile([128, NLOC], BF, name=f"hT{nm}", tag="ht")
                Th_max = (Tq_max + 1) // 2
                qrr = [0]
                for q in range(NQ):
                    nT = T_quad[q]
                    nA = (nT + 1) // 2
                    msgs_l = []
                    for h0_, hn in ((0, nA), (nA, nT)):
                        cnt = hn - h0_
                        if cnt <= 0:
                            continue
                        mt_ = msgsp.tile(
                            [128, Th_max, HID], BF, name=f"msgs{nm}", tag="msgs"
                        )
                        nc.gpsimd.dma_gather(
                            mt_[:, :cnt, :],
                            cc_out[:],
                            idx_sb[
                                :,
                                (TB[WPQ * q] + h0_) * 8 : (TB[WPQ * q] + hn) * 8,
                            ],
                            cnt * 128,
                            gs_reg(cnt * 128),
                            HID,
                            single_packet=False,
                            queue_num=qrr[0] % 4,
                        )
                        qrr[0] += 1
                        msgs_l.append((h0_, hn, mt_))

                    def msel(lt):
                        for h0_, hn, mt_ in msgs_l:
                            if h0_ <= lt < hn:
                                return mt_[:, lt - h0_, :]
                        raise AssertionError
                    ps = psump.tile([128, HID], F32, name=f"segps{nm}", tag="seg", bufs=3)
                    for wi in range(WPQ):
                        wdx = WPQ * q + wi
                        tw = int(T_w[wdx])
                        base = int(TB[wdx] - TB[WPQ * q])
                        for t in range(tw):
                            nc.tensor.matmul(
                                ps[32 * wi : 32 * (wi + 1), :],
                                m_sb[:, int(TB[wdx]) + t, :],
                                msel(base + t),
                                start=(t == 0),
                                stop=(t == tw - 1),
                                tile_position=(0, 32 * wi),
                            )
                    # finalize: out = beta*h + agg ; out /= max(||out||, eps); +b; relu
                    res = workp.tile([128, HID], F32, name=f"res{nm}", tag="res")
                    nc.vector.scalar_tensor_tensor(
                        res[:], h_in[q][:], float(BETA), ps[:], OP.mult, OP.add
                    )
                    sq = workp.tile([128, HID], F32, name=f"sq{nm}", tag="sq")
                    nsq = smallp.tile([128, 1], F32, name=f"nsq{nm}", tag="nsq")
                    nc.scalar.activation(sq[:], res[:], AF.Square, accum_out=nsq[:])
                    nrm = smallp.tile([128, 1], F32, name=f"nrm{nm}", tag="nrm")
                    nc.scalar.activation(nrm[:], nsq[:], AF.Sqrt)
                    nrm2 = smallp.tile([128, 1], F32, name=f"nrm2{nm}", tag="nrm2")
                    nc.vector.scalar_tensor_tensor(
                        nrm2[:], nrm[:], float(EPS), nrm[:], OP.max, OP.max
                    )
                    rnm = smallp.tile([128, 1], F32, name=f"rnm{nm}", tag="rnm")
                    nc.vector.reciprocal(rnm[:], nrm2[:])
                    scaled = workp.tile([128, HID], F32, name=f"scl{nm}", tag="scl")
                    nc.scalar.mul(scaled[:], res[:], rnm[:])
                    hosum = workp.tile([128, HID], F32, name=f"hos{nm}", tag="hos")
                    nc.vector.scalar_tensor_tensor(
                        hosum[:], scaled[:], 1.0, bt[:], OP.mult, OP.add
                    )
                    ho = h_out[q]
                    nc.scalar.activation(ho[:], hosum[:], AF.Relu)
                    if keep_bf is None:
                        hb = workp.tile([128, 128], BF, name=f"hb{nm}", tag="hb")
                    else:
                        hb = keep_bf[:, q, :]
                    nc.scalar.copy(hb[:], ho[:])
                    pst = psump.tile(
                        [128, 128], BF, name=f"trps{nm}", tag="tr", bufs=2
                    )
                    nc.tensor.transpose(pst[:], hb[:], ident[:])
                    nc.scalar.copy(hT_out[:, 128 * q : 128 * (q + 1)], pst[:])
                return h_out, hT_out

            h1, hT1 = gcn_layer(0, h0, hT0, gcw[0], gcb[0], "e1")
            h2, hT2 = gcn_layer(1, h1, hT1, gcw[1], gcb[1], "e2")

            # structure decoder layer; keep bf16 copy of s for the AllGather
            s_bf = suppp.tile([128, JJ, HID], BF, name="s_bf", tag="sbf", bufs=1)
            _s, sT = gcn_layer(4, h2, hT2, gcw[4], gcb[4], "s", keep_bf=s_bf)
            s_cc_in = dramp.tile([NLOC, HID], BF, name="s_ccin", tag="ccin")
            nc.sync.dma_start(
                s_cc_in[:].rearrange("(p j) f -> p j f", p=128), s_bf[:]
            )
            s_cc_out = dramp.tile(
                [N, HID], BF, name="s_ccout", tag="ccout", addr_space="Shared"
            )
            nc.gpsimd.collective_compute(
                "AllGather",
                OP.bypass,
                replica_groups=[list(range(NCORES))],
                ins=[s_cc_in.opt()],
                outs=[s_cc_out.opt()],
            )
            sfT = bigp.tile([128, N], BF, name="sfT", tag="big")
            nc.sync.dma_start(sfT[:], s_cc_out[:], transpose=True)

            # attribute decoder branch
            a1, aT1 = gcn_layer(2, h2, hT2, gcw[2], gcb[2], "a1")
            a2, aT2 = gcn_layer(3, a1, aT1, gcw[3], gcb[3], "a2")

            # x_hat = a2 @ attrW + attrb
            for j in range(NQ):
                ps = psump.tile([128, FEAT], F32, name="xps", tag="mm512", bufs=3)
                nc.tensor.matmul(
                    ps[:],
                    aT2[:, 128 * j : 128 * (j + 1)],
                    attrw[:],
                    start=True,
                    stop=True,
                )
                xst = stagep.tile([128, FEAT], F32, name="xst", tag="xst")
                nc.vector.tensor_add(xst[:], ps[:], attrb[:])
                nc.sync.dma_start(xhat_out[128 * j : 128 * (j + 1), :], xst[:])

            # a_hat = s @ s_full.T  (columns unpermuted during PSUM->SBUF copy)
            cp_flip = 0
            for j in range(NQ):
                for cb in range(8):
                    st = stagep.tile([128, NLOC], F32, name="ast", tag="ast")
                    # natural cols n = 128*jj + 32*m + pr ; st viewed [pr, jj]
                    stv = st[:].rearrange("p (jj pp) -> p pp jj", jj=JJ)
                    for m in range(4):
                        ps = psump.tile(
                            [128, 512], F32, name="aps", tag="mm512", bufs=3
                        )
                        nc.tensor.matmul(
                            ps[:],
                            sT[:, 128 * j : 128 * (j + 1)],
                            sfT[:, 2048 * cb + 512 * m : 2048 * cb + 512 * (m + 1)],
                            start=True,
                            stop=True,
                        )
                        psv = ps[:].rearrange("p (pr jj) -> p pr jj", pr=32)
                        if cp_flip % 2 == 0:
                            nc.vector.tensor_copy(
                                stv[:, 32 * m : 32 * (m + 1), :], psv
                            )
                        else:
                            nc.scalar.copy(stv[:, 32 * m : 32 * (m + 1), :], psv)
                        cp_flip += 1
                    nc.sync.dma_start(
                        ahat_out[
                            128 * j : 128 * (j + 1), 2048 * cb : 2048 * (cb + 1)
                        ],
                        st[:],
                    )

    nc.compile()
    return nc


def prep_inputs(x, prep, core):
    """Per-core input map (numpy arrays matching declared dtypes)."""
    xl = np.asarray(x[core * NLOC : (core + 1) * NLOC], np.float32)  # [2048, 512]
    xt = np.ascontiguousarray(
        xl.T.reshape(4, 128, NLOC).transpose(1, 0, 2)
    ).astype(BF_NP)
    mt = np.ascontiguousarray(
        prep["M"][core].reshape(128, prep["T_total"] * RW)
    ).astype(BF_NP)
    idx = np.ascontiguousarray(prep["idx16"][core])
    return dict(xt=xt, mt=mt, idx=idx)


def shared_inputs(enc_lin_W, enc_lin_b, enc_gc1_W, enc_gc1_b, enc_gc2_W, enc_gc2_b,
                  attr_gc1_W, attr_gc1_b, attr_gc2_W, attr_gc2_b,
                  attr_lin_W, attr_lin_b, str_gc1_W, str_gc1_b):
    def bcast(b, width):
        return np.ascontiguousarray(
            np.broadcast_to(np.asarray(b, np.float32), (128, width))
        )

    d = dict(
        encw=np.ascontiguousarray(
            np.asarray(enc_lin_W, np.float32).reshape(4, 128, HID).transpose(1, 0, 2)
        ).astype(BF_NP),
        encb=bcast(enc_lin_b, HID),
        attrw=np.asarray(attr_lin_W, np.float32).astype(BF_NP),
        attrb=bcast(attr_lin_b, FEAT),
        ident=np.eye(128, dtype=BF_NP),
    )
    gws = [enc_gc1_W, enc_gc2_W, attr_gc1_W, attr_gc2_W, str_gc1_W]
    gbs = [enc_gc1_b, enc_gc2_b, attr_gc1_b, attr_gc2_b, str_gc1_b]
    for l in range(5):
        d[f"gcw{l}"] = np.asarray(gws[l], np.float32).astype(BF_NP)
        d[f"gcb{l}"] = bcast(gbs[l], HID)
    return d


_BUILD_CACHE = {}


def kernel(x, edge_index, edge_weight,
           enc_lin_W, enc_lin_b, enc_gc1_W, enc_gc1_b, enc_gc2_W, enc_gc2_b,
           attr_gc1_W, attr_gc1_b, attr_gc2_W, attr_gc2_b, attr_lin_W, attr_lin_b,
           str_gc1_W, str_gc1_b, _trace=False):
    x = np.asarray(x, np.float32)
    edge_index = np.asarray(edge_index)
    edge_weight = np.asarray(edge_weight, np.float32)

    prep = prep_graph(edge_index, edge_weight)
    ck = prep["M"].tobytes()[:0]  # program depends only on T_w/TB
    key = (tuple(prep["T_w"].tolist()),)
    if key not in _BUILD_CACHE:
        _BUILD_CACHE[key] = build_program(prep["T_w"], prep["TB"], prep["T_total"])
    nc = _BUILD_CACHE[key]

    shared = shared_inputs(
        enc_lin_W, enc_lin_b, enc_gc1_W, enc_gc1_b, enc_gc2_W, enc_gc2_b,
        attr_gc1_W, attr_gc1_b, attr_gc2_W, attr_gc2_b, attr_lin_W, attr_lin_b,
        str_gc1_W, str_gc1_b,
    )
    in_maps = []
    for c in range(NCORES):
        m = prep_inputs(x, prep, c)
        m.update(shared)
        in_maps.append(m)

    res = run_bass_kernel_spmd(nc, in_maps, list(range(NCORES)), trace=_trace)

    a_hat = np.concatenate([res.results[c]["ahat"] for c in range(NCORES)], axis=0)
    x_hat = np.concatenate([res.results[c]["xhat"] for c in range(NCORES)], axis=0)
    if _trace:
        kernel.last_exec_time_ns = res.exec_time_ns
        kernel.last_results = res
    return (a_hat, x_hat)


# revision 8
# speedup vs baseline: 1.5636x; 1.0315x over previous
"""Trainium2 Bass kernel for nn_DominantNeigh (GCN autoencoder, DOMINANT-style).

Distribution: nodes row-sharded across 8 NeuronCores (2048 rows/core); edges
partitioned by destination row. Per GCN layer each core computes its support
shard (h @ W in bf16 on the PE), all-gathers support into a DRAM buffer
(p-major row permutation so the SBUF->DRAM store is one contiguous DMA), then
edge messages are fetched with dma_gather (128 edges land on 128 partitions)
and the weighted segment-sum is a PE matmul against host-precomputed one-hot
x weight scatter matrices, accumulated in PSUM in 32-row dst windows (4
windows per PSUM tile via col-group tile_position). Residual + L2 row-norm +
bias + relu run on DVE/ACT from PSUM. a_hat = s @ s.T is row-sharded
([2048,16384] f32 per core, the dominant HBM write); x_hat via attr decoder.
"""

import sys

sys.path.insert(0, "/opt/trn_rl_repo")

import numpy as np
import ml_dtypes

from concourse import bacc, bass, tile
from concourse import mybir
from concourse.bass_utils import run_bass_kernel_spmd

# problem shape (hardcoded; kernel.py must be self-contained)
N, FEAT, HID, E = 16384, 512, 128, 524288
BETA, EPS = 0.001, 1e-12
NCORES = 8
NLOC = N // NCORES  # 2048
RW = 32  # dst rows per window (one 32-col PE group)
NWIN = NLOC // RW  # 64 windows per core
WPQ = 4  # windows per PSUM quad tile (128 rows)
NQ = NWIN // WPQ  # 16 quads per core
JJ = NLOC // 128  # 16 sub-rows per partition in p-major shard layout

BF = mybir.dt.bfloat16
F32 = mybir.dt.float32
I16 = mybir.dt.int16
BF_NP = ml_dtypes.bfloat16
AF = mybir.ActivationFunctionType
OP = mybir.AluOpType


def prep_graph(edge_index, edge_weight):
    """Host-side: dedup (dst,src) edges, partition by dst core/window, sort by
    gather address, build per-core scatter matrices M and gather indices.

    Returns dict with T_w [NWIN], TB [NWIN+1], T_total, M [NCORES,128,Tt,RW]
    f32, idx16 [NCORES,128,Tt*8] int16 (replicated over partition groups).
    """
    dst = np.asarray(edge_index[0], dtype=np.int64)
    src = np.asarray(edge_index[1], dtype=np.int64)
    w = np.asarray(edge_weight, dtype=np.float64) * (1.0 - BETA)

    # dedup (dst, src): segment_sum adds duplicate edges
    key = dst * N + src
    order = np.argsort(key, kind="stable")
    ks, ws = key[order], w[order]
    newgrp = np.ones(len(ks), bool)
    newgrp[1:] = ks[1:] != ks[:-1]
    gid = np.cumsum(newgrp) - 1
    wsum = np.zeros(int(gid[-1]) + 1)
    np.add.at(wsum, gid, ws)
    ukey = ks[newgrp]
    udst, usrc = ukey // N, ukey % N
    # p-major permuted row of src in the all-gathered buffer:
    # shard row r = (n%128)*JJ + n//128 for local n
    sloc = usrc % NLOC
    uv = (usrc // NLOC) * NLOC + (sloc % 128) * JJ + sloc // 128
    core = udst // NLOC
    dloc = udst % NLOC
    winloc = dloc // RW
    roff = dloc % RW

    # order by (core, window, gather address) - address order gives HBM locality
    o2 = np.lexsort((uv, winloc, core))
    core, winloc, roff, uv, wsum = core[o2], winloc[o2], roff[o2], uv[o2], wsum[o2]

    counts = np.zeros((NCORES, NWIN), np.int64)
    np.add.at(counts, (core, winloc), 1)
    T_w = np.maximum(1, -(-counts.max(axis=0) // 128))  # tiles per window
    TB = np.zeros(NWIN + 1, np.int64)
    TB[1:] = np.cumsum(T_w)
    T_total = int(TB[-1])

    # position of each entry within its (core, window) group
    gkey = core * NWIN + winloc
    newg = np.ones(len(gkey), bool)
    newg[1:] = gkey[1:] != gkey[:-1]
    gidx = np.cumsum(newg) - 1
    starts = np.flatnonzero(newg)
    pos = np.arange(len(gkey)) - starts[gidx]

    q = winloc // WPQ
    slot = (TB[winloc] - TB[WPQ * q]) * 128 + pos  # call-local gather slot
    tile_g = TB[winloc] + pos // 128
    part = pos % 128

    M = np.zeros((NCORES, 128, T_total, RW), np.float32)
    M[core, part, tile_g, roff] = wsum

    idx16 = np.zeros((NCORES, 16, T_total * 8), np.int16)
    colv = TB[WPQ * q] * 8 + slot // 16
    idx16[core, slot % 16, colv] = uv.astype(np.int16)
    idx16 = np.tile(idx16, (1, 8, 1))  # replicate to 128 partitions

    return dict(T_w=T_w, TB=TB, T_total=T_total, M=M, idx16=idx16)


def build_program(T_w, TB, T_total):
    """Build the SPMD Bass program (shared by all 8 cores)."""
    T_quad = [int(TB[WPQ * (q + 1)] - TB[WPQ * q]) for q in range(NQ)]
    Tq_max = max(T_quad)

    nc = bacc.Bacc(num_devices=NCORES, num_swdge_queues=4)

    xt_in = nc.dram_tensor("xt", [128, 4, NLOC], BF, kind="ExternalInput")
    m_in = nc.dram_tensor("mt", [128, T_total * RW], BF, kind="ExternalInput")
    idx_in = nc.dram_tensor("idx", [128, T_total * 8], I16, kind="ExternalInput")
    encw_in = nc.dram_tensor("encw", [128, 4, HID], BF, kind="ExternalInput")
    encb_in = nc.dram_tensor("encb", [128, HID], F32, kind="ExternalInput")
    gcw_in = [
        nc.dram_tensor(f"gcw{l}", [HID, HID], BF, kind="ExternalInput")
        for l in range(5)
    ]
    gcb_in = [
        nc.dram_tensor(f"gcb{l}", [128, HID], F32, kind="ExternalInput")
        for l in range(5)
    ]
    attrw_in = nc.dram_tensor("attrw", [HID, FEAT], BF, kind="ExternalInput")
    attrb_in = nc.dram_tensor("attrb", [128, FEAT], F32, kind="ExternalInput")
    ident_in = nc.dram_tensor("ident", [128, 128], BF, kind="ExternalInput")
    ahat_out = nc.dram_tensor("ahat", [NLOC, N], F32, kind="ExternalOutput")
    xhat_out = nc.dram_tensor("xhat", [NLOC, FEAT], F32, kind="ExternalOutput")

    with tile.TileContext(nc) as tc:
        with (
            tc.tile_pool(name="const", bufs=1) as constp,
            tc.tile_pool(name="big", bufs=1) as bigp,
            tc.tile_pool(name="hp", bufs=44) as hp,
            tc.tile_pool(name="htp", bufs=4) as htp,
            tc.tile_pool(name="msgsp", bufs=10) as msgsp,
            tc.tile_pool(name="workp", bufs=3) as workp,
            tc.tile_pool(name="smallp", bufs=8) as smallp,
            tc.tile_pool(name="suppp", bufs=2) as suppp,
            tc.tile_pool(name="stagep", bufs=2) as stagep,
            tc.tile_pool(name="psum", bufs=1, space="PSUM") as psump,
            tc.tile_pool(name="dramp", bufs=2, space="DRAM") as dramp,
        ):
            # ---- constants ----
            ident = constp.tile([128, 128], BF)
            nc.sync.dma_start(ident[:], ident_in[:])
            encw = constp.tile([128, 4, HID], BF)
            nc.sync.dma_start(encw[:], encw_in[:])
            encb = constp.tile([128, HID], F32)
            nc.sync.dma_start(encb[:], encb_in[:])
            gcw, gcb = [], []
            for l in range(5):
                wt = constp.tile([HID, HID], BF, name=f"gcw{l}")
                nc.sync.dma_start(wt[:], gcw_in[l][:])
                gcw.append(wt)
                bt = constp.tile([128, HID], F32, name=f"gcb{l}")
                nc.sync.dma_start(bt[:], gcb_in[l][:])
                gcb.append(bt)
            attrw = constp.tile([HID, FEAT], BF)
            nc.sync.dma_start(attrw[:], attrw_in[:])
            attrb = constp.tile([128, FEAT], F32)
            nc.sync.dma_start(attrb[:], attrb_in[:])
            m_sb = constp.tile([128, T_total, RW], BF)
            nc.sync.dma_start(m_sb[:], m_in[:].rearrange("p (t r) -> p t r", r=RW))
            idx_sb = constp.tile([128, T_total * 8], I16)
            nc.sync.dma_start(idx_sb[:], idx_in[:])
            xt = bigp.tile([128, 4, NLOC], BF, tag="big")
            nc.sync.dma_start(xt[:], xt_in[:])

            # cached scalar registers for dma_gather num_idxs
            reg_cache = {}

            def gs_reg(v):
                if v not in reg_cache:
                    reg_cache[v] = nc.gpsimd.to_reg(v)
                return reg_cache[v]

            def new_hgen(nm):
                return [
                    hp.tile([128, 128], F32, name=f"h_{nm}_{j}", tag="h")
                    for j in range(NQ)
                ]

            # ---- encoder linear: h0 = x @ encW + encb (node-major) ----
            h0 = new_hgen("h0")
            hT0 = htp.tile([128, NLOC], BF, name="hT0", tag="ht")
            for j in range(NQ):
                ps = psump.tile([128, HID], F32, name="encps", tag="mm512", bufs=3)
                for k in range(4):
                    nc.tensor.matmul(
                        ps[:],
                        xt[:, k, 128 * j : 128 * (j + 1)],
                        encw[:, k, :],
                        start=(k == 0),
                        stop=(k == 3),
                    )
                nc.vector.tensor_add(h0[j][:], ps[:], encb[:])
                hb = workp.tile([128, 128], BF, name="hb0", tag="hb")
                nc.scalar.copy(hb[:], h0[j][:])
                pst = psump.tile([128, 128], BF, name="trps0", tag="tr", bufs=2)
                nc.tensor.transpose(pst[:], hb[:], ident[:])
                nc.scalar.copy(hT0[:, 128 * j : 128 * (j + 1)], pst[:])

            # ---- one GCN layer ----
            def gcn_layer(lidx, h_in, hT_in, Wt, bt, nm, keep_bf=None):
                # support = h @ W (node-major, bf16), store p-major to DRAM
                supp = suppp.tile([128, JJ, HID], BF, name=f"supp{nm}", tag="supp")
                for j in range(NQ):
                    ps = psump.tile(
                        [128, HID], F32, name=f"sps{nm}", tag="mm512", bufs=3
                    )
                    nc.tensor.matmul(
                        ps[:],
                        hT_in[:, 128 * j : 128 * (j + 1)],
                        Wt[:],
                        start=True,
                        stop=True,
                    )
                    nc.vector.tensor_copy(supp[:, j, :], ps[:])
                cc_in = dramp.tile([NLOC, HID], BF, name=f"ccin{nm}", tag="ccin")
                nc.sync.dma_start(
                    cc_in[:].rearrange("(p j) f -> p j f", p=128), supp[:]
                )
                cc_out = dramp.tile(
                    [N, HID], BF, name=f"ccout{nm}", tag="ccout", addr_space="Shared"
                )
                nc.gpsimd.collective_compute(
                    "AllGather",
                    OP.bypass,
                    replica_groups=[list(range(NCORES))],
                    ins=[cc_in.opt()],
                    outs=[cc_out.opt()],
                )

                h_out = new_hgen(nm)
                hT_out = htp.tile([128, NLOC], BF, name=f"hT{nm}", tag="ht")
                Th_max = (Tq_max + 3) // 4
                qrr = [0]
                for q in range(NQ):
                    nT = T_quad[q]
                    nA = (nT + 3) // 4
                    bounds = [(i * nA, min((i + 1) * nA, nT)) for i in range(4)]
                    msgs_l = []
                    for h0_, hn in bounds:
                        cnt = hn - h0_
                        if cnt <= 0:
                            continue
                        mt_ = msgsp.tile(
                            [128, Th_max, HID], BF, name=f"msgs{nm}", tag="msgs"
                        )
                        nc.gpsimd.dma_gather(
                            mt_[:, :cnt, :],
                            cc_out[:],
                            idx_sb[
                                :,
                                (TB[WPQ * q] + h0_) * 8 : (TB[WPQ * q] + hn) * 8,
                            ],
                            cnt * 128,
                            gs_reg(cnt * 128),
                            HID,
                            single_packet=False,
                            queue_num=qrr[0] % 4,
                        )
                        qrr[0] += 1
                        msgs_l.append((h0_, hn, mt_))

                    def msel(lt):
                        for h0_, hn, mt_ in msgs_l:
                            if h0_ <= lt < hn:
                                return mt_[:, lt - h0_, :]
                        raise AssertionError
                    ps = psump.tile([128, HID], F32, name=f"segps{nm}", tag="seg", bufs=3)
                    for wi in range(WPQ):
                        wdx = WPQ * q + wi
                        tw = int(T_w[wdx])
                        base = int(TB[wdx] - TB[WPQ * q])
                        for t in range(tw):
                            nc.tensor.matmul(
                                ps[32 * wi : 32 * (wi + 1), :],
                                m_sb[:, int(TB[wdx]) + t, :],
                                msel(base + t),
                                start=(t == 0),
                                stop=(t == tw - 1),
                                tile_position=(0, 32 * wi),
                            )
                    # finalize: out = beta*h + agg ; out /= max(||out||, eps); +b; relu
                    res = workp.tile([128, HID], F32, name=f"res{nm}", tag="res")
                    nc.vector.scalar_tensor_tensor(
                        res[:], h_in[q][:], float(BETA), ps[:], OP.mult, OP.add
                    )
                    sq = workp.tile([128, HID], F32, name=f"sq{nm}", tag="sq")
                    nsq = smallp.tile([128, 1], F32, name=f"nsq{nm}", tag="nsq")
                    nc.scalar.activation(sq[:], res[:], AF.Square, accum_out=nsq[:])
                    nrm = smallp.tile([128, 1], F32, name=f"nrm{nm}", tag="nrm")
                    nc.scalar.activation(nrm[:], nsq[:], AF.Sqrt)
                    nrm2 = smallp.tile([128, 1], F32, name=f"nrm2{nm}", tag="nrm2")
                    nc.vector.scalar_tensor_tensor(
                        nrm2[:], nrm[:], float(EPS), nrm[:], OP.max, OP.max
                    )
                    rnm = smallp.tile([128, 1], F32, name=f"rnm{nm}", tag="rnm")
                    nc.vector.reciprocal(rnm[:], nrm2[:])
                    scaled = workp.tile([128, HID], F32, name=f"scl{nm}", tag="scl")
                    nc.scalar.mul(scaled[:], res[:], rnm[:])
                    hosum = workp.tile([128, HID], F32, name=f"hos{nm}", tag="hos")
                    nc.vector.scalar_tensor_tensor(
                        hosum[:], scaled[:], 1.0, bt[:], OP.mult, OP.add
                    )
                    ho = h_out[q]
                    nc.scalar.activation(ho[:], hosum[:], AF.Relu)
                    if keep_bf is None:
                        hb = workp.tile([128, 128], BF, name=f"hb{nm}", tag="hb")
                    else:
                        hb = keep_bf[:, q, :]
                    nc.scalar.copy(hb[:], ho[:])
                    pst = psump.tile(
                        [128, 128], BF, name=f"trps{nm}", tag="tr", bufs=2
                    )
                    nc.tensor.transpose(pst[:], hb[:], ident[:])
                    nc.scalar.copy(hT_out[:, 128 * q : 128 * (q + 1)], pst[:])
                return h_out, hT_out

            h1, hT1 = gcn_layer(0, h0, hT0, gcw[0], gcb[0], "e1")
            h2, hT2 = gcn_layer(1, h1, hT1, gcw[1], gcb[1], "e2")

            # structure decoder layer; keep bf16 copy of s for the AllGather
            s_bf = suppp.tile([128, JJ, HID], BF, name="s_bf", tag="sbf", bufs=1)
            _s, sT = gcn_layer(4, h2, hT2, gcw[4], gcb[4], "s", keep_bf=s_bf)
            s_cc_in = dramp.tile([NLOC, HID], BF, name="s_ccin", tag="ccin")
            nc.sync.dma_start(
                s_cc_in[:].rearrange("(p j) f -> p j f", p=128), s_bf[:]
            )
            s_cc_out = dramp.tile(
                [N, HID], BF, name="s_ccout", tag="ccout", addr_space="Shared"
            )
            nc.gpsimd.collective_compute(
                "AllGather",
                OP.bypass,
                replica_groups=[list(range(NCORES))],
                ins=[s_cc_in.opt()],
                outs=[s_cc_out.opt()],
            )
            sfT = bigp.tile([128, N], BF, name="sfT", tag="big")
            nc.sync.dma_start(sfT[:], s_cc_out[:], transpose=True)

            # attribute decoder branch
            a1, aT1 = gcn_layer(2, h2, hT2, gcw[2], gcb[2], "a1")
            a2, aT2 = gcn_layer(3, a1, aT1, gcw[3], gcb[3], "a2")

            # x_hat = a2 @ attrW + attrb
            for j in range(NQ):
                ps = psump.tile([128, FEAT], F32, name="xps", tag="mm512", bufs=3)
                nc.tensor.matmul(
                    ps[:],
                    aT2[:, 128 * j : 128 * (j + 1)],
                    attrw[:],
                    start=True,
                    stop=True,
                )
                xst = stagep.tile([128, FEAT], F32, name="xst", tag="xst")
                nc.vector.tensor_add(xst[:], ps[:], attrb[:])
                nc.sync.dma_start(xhat_out[128 * j : 128 * (j + 1), :], xst[:])

            # a_hat = s @ s_full.T  (columns unpermuted during PSUM->SBUF copy)
            cp_flip = 0
            for j in range(NQ):
                for cb in range(8):
                    st = stagep.tile([128, NLOC], F32, name="ast", tag="ast")
                    # natural cols n = 128*jj + 32*m + pr ; st viewed [pr, jj]
                    stv = st[:].rearrange("p (jj pp) -> p pp jj", jj=JJ)
                    for m in range(4):
                        ps = psump.tile(
                            [128, 512], F32, name="aps", tag="mm512", bufs=3
                        )
                        nc.tensor.matmul(
                            ps[:],
                            sT[:, 128 * j : 128 * (j + 1)],
                            sfT[:, 2048 * cb + 512 * m : 2048 * cb + 512 * (m + 1)],
                            start=True,
                            stop=True,
                        )
                        psv = ps[:].rearrange("p (pr jj) -> p pr jj", pr=32)
                        if cp_flip % 2 == 0:
                            nc.vector.tensor_copy(
                                stv[:, 32 * m : 32 * (m + 1), :], psv
                            )
                        else:
                            nc.scalar.copy(stv[:, 32 * m : 32 * (m + 1), :], psv)
                        cp_flip += 1
                    nc.sync.dma_start(
                        ahat_out[
                            128 * j : 128 * (j + 1), 2048 * cb : 2048 * (cb + 1)
                        ],
                        st[:],
                    )

    nc.compile()
    return nc


def prep_inputs(x, prep, core):
    """Per-core input map (numpy arrays matching declared dtypes)."""
    xl = np.asarray(x[core * NLOC : (core + 1) * NLOC], np.float32)  # [2048, 512]
    xt = np.ascontiguousarray(
        xl.T.reshape(4, 128, NLOC).transpose(1, 0, 2)
    ).astype(BF_NP)
    mt = np.ascontiguousarray(
        prep["M"][core].reshape(128, prep["T_total"] * RW)
    ).astype(BF_NP)
    idx = np.ascontiguousarray(prep["idx16"][core])
    return dict(xt=xt, mt=mt, idx=idx)


def shared_inputs(enc_lin_W, enc_lin_b, enc_gc1_W, enc_gc1_b, enc_gc2_W, enc_gc2_b,
                  attr_gc1_W, attr_gc1_b, attr_gc2_W, attr_gc2_b,
                  attr_lin_W, attr_lin_b, str_gc1_W, str_gc1_b):
    def bcast(b, width):
        return np.ascontiguousarray(
            np.broadcast_to(np.asarray(b, np.float32), (128, width))
        )

    d = dict(
        encw=np.ascontiguousarray(
            np.asarray(enc_lin_W, np.float32).reshape(4, 128, HID).transpose(1, 0, 2)
        ).astype(BF_NP),
        encb=bcast(enc_lin_b, HID),
        attrw=np.asarray(attr_lin_W, np.float32).astype(BF_NP),
        attrb=bcast(attr_lin_b, FEAT),
        ident=np.eye(128, dtype=BF_NP),
    )
    gws = [enc_gc1_W, enc_gc2_W, attr_gc1_W, attr_gc2_W, str_gc1_W]
    gbs = [enc_gc1_b, enc_gc2_b, attr_gc1_b, attr_gc2_b, str_gc1_b]
    for l in range(5):
        d[f"gcw{l}"] = np.asarray(gws[l], np.float32).astype(BF_NP)
        d[f"gcb{l}"] = bcast(gbs[l], HID)
    return d


_BUILD_CACHE = {}


def kernel(x, edge_index, edge_weight,
           enc_lin_W, enc_lin_b, enc_gc1_W, enc_gc1_b, enc_gc2_W, enc_gc2_b,
           attr_gc1_W, attr_gc1_b, attr_gc2_W, attr_gc2_b, attr_lin_W, attr_lin_b,
           str_gc1_W, str_gc1_b, _trace=False):
    x = np.asarray(x, np.float32)
    edge_index = np.asarray(edge_index)
    edge_weight = np.asarray(edge_weight, np.float32)

    prep = prep_graph(edge_index, edge_weight)
    ck = prep["M"].tobytes()[:0]  # program depends only on T_w/TB
    key = (tuple(prep["T_w"].tolist()),)
    if key not in _BUILD_CACHE:
        _BUILD_CACHE[key] = build_program(prep["T_w"], prep["TB"], prep["T_total"])
    nc = _BUILD_CACHE[key]

    shared = shared_inputs(
        enc_lin_W, enc_lin_b, enc_gc1_W, enc_gc1_b, enc_gc2_W, enc_gc2_b,
        attr_gc1_W, attr_gc1_b, attr_gc2_W, attr_gc2_b, attr_lin_W, attr_lin_b,
        str_gc1_W, str_gc1_b,
    )
    in_maps = []
    for c in range(NCORES):
        m = prep_inputs(x, prep, c)
        m.update(shared)
        in_maps.append(m)

    res = run_bass_kernel_spmd(nc, in_maps, list(range(NCORES)), trace=_trace)

    a_hat = np.concatenate([res.results[c]["ahat"] for c in range(NCORES)], axis=0)
    x_hat = np.concatenate([res.results[c]["xhat"] for c in range(NCORES)], axis=0)
    if _trace:
        kernel.last_exec_time_ns = res.exec_time_ns
        kernel.last_results = res
    return (a_hat, x_hat)


# revision 9
# speedup vs baseline: 1.7545x; 1.1221x over previous
"""Trainium2 Bass kernel for nn_DominantNeigh (GCN autoencoder, DOMINANT-style).

Distribution: nodes row-sharded across 8 NeuronCores (2048 rows/core); edges
partitioned by destination row. Per GCN layer each core computes its support
shard (h @ W in bf16 on the PE), all-gathers support into a DRAM buffer
(p-major row permutation so the SBUF->DRAM store is one contiguous DMA), then
edge messages are fetched with dma_gather (128 edges land on 128 partitions)
and the weighted segment-sum is a PE matmul against host-precomputed one-hot
x weight scatter matrices, accumulated in PSUM in 32-row dst windows (4
windows per PSUM tile via col-group tile_position). Residual + L2 row-norm +
bias + relu run on DVE/ACT from PSUM. a_hat = s @ s.T is row-sharded
([2048,16384] f32 per core, the dominant HBM write); x_hat via attr decoder.
"""

import sys

sys.path.insert(0, "/opt/trn_rl_repo")

import numpy as np
import ml_dtypes

from concourse import bacc, bass, tile
from concourse import mybir
from concourse.bass_utils import run_bass_kernel_spmd

# problem shape (hardcoded; kernel.py must be self-contained)
N, FEAT, HID, E = 16384, 512, 128, 524288
BETA, EPS = 0.001, 1e-12
NCORES = 8
NLOC = N // NCORES  # 2048
RW = 32  # dst rows per window (one 32-col PE group)
NWIN = NLOC // RW  # 64 windows per core
WPQ = 4  # windows per PSUM quad tile (128 rows)
NQ = NWIN // WPQ  # 16 quads per core
JJ = NLOC // 128  # 16 sub-rows per partition in p-major shard layout

BF = mybir.dt.bfloat16
F32 = mybir.dt.float32
I16 = mybir.dt.int16
BF_NP = ml_dtypes.bfloat16
AF = mybir.ActivationFunctionType
OP = mybir.AluOpType


def prep_graph(edge_index, edge_weight):
    """Host-side: dedup (dst,src) edges, partition by dst core/window, sort by
    gather address, build per-core scatter matrices M and gather indices.

    Returns dict with T_w [NWIN], TB [NWIN+1], T_total, M [NCORES,128,Tt,RW]
    f32, idx16 [NCORES,128,Tt*8] int16 (replicated over partition groups).
    """
    dst = np.asarray(edge_index[0], dtype=np.int64)
    src = np.asarray(edge_index[1], dtype=np.int64)
    w = np.asarray(edge_weight, dtype=np.float64) * (1.0 - BETA)

    # dedup (dst, src): segment_sum adds duplicate edges
    key = dst * N + src
    order = np.argsort(key, kind="stable")
    ks, ws = key[order], w[order]
    newgrp = np.ones(len(ks), bool)
    newgrp[1:] = ks[1:] != ks[:-1]
    gid = np.cumsum(newgrp) - 1
    wsum = np.zeros(int(gid[-1]) + 1)
    np.add.at(wsum, gid, ws)
    ukey = ks[newgrp]
    udst, usrc = ukey // N, ukey % N
    # p-major permuted row of src in the all-gathered buffer:
    # shard row r = (n%128)*JJ + n//128 for local n
    sloc = usrc % NLOC
    uv = (usrc // NLOC) * NLOC + (sloc % 128) * JJ + sloc // 128
    core = udst // NLOC
    dloc = udst % NLOC
    winloc = dloc // RW
    roff = dloc % RW

    # order by (core, window, gather address) - address order gives HBM locality
    o2 = np.lexsort((uv, winloc, core))
    core, winloc, roff, uv, wsum = core[o2], winloc[o2], roff[o2], uv[o2], wsum[o2]

    counts = np.zeros((NCORES, NWIN), np.int64)
    np.add.at(counts, (core, winloc), 1)
    T_w = np.maximum(1, -(-counts.max(axis=0) // 128))  # tiles per window
    TB = np.zeros(NWIN + 1, np.int64)
    TB[1:] = np.cumsum(T_w)
    T_total = int(TB[-1])

    # position of each entry within its (core, window) group
    gkey = core * NWIN + winloc
    newg = np.ones(len(gkey), bool)
    newg[1:] = gkey[1:] != gkey[:-1]
    gidx = np.cumsum(newg) - 1
    starts = np.flatnonzero(newg)
    pos = np.arange(len(gkey)) - starts[gidx]

    q = winloc // WPQ
    slot = (TB[winloc] - TB[WPQ * q]) * 128 + pos  # call-local gather slot
    tile_g = TB[winloc] + pos // 128
    part = pos % 128

    M = np.zeros((NCORES, 128, T_total, RW), np.float32)
    M[core, part, tile_g, roff] = wsum

    idx16 = np.zeros((NCORES, 16, T_total * 8), np.int16)
    colv = TB[WPQ * q] * 8 + slot // 16
    idx16[core, slot % 16, colv] = uv.astype(np.int16)
    idx16 = np.tile(idx16, (1, 8, 1))  # replicate to 128 partitions

    return dict(T_w=T_w, TB=TB, T_total=T_total, M=M, idx16=idx16)


def build_program(T_w, TB, T_total):
    """Build the SPMD Bass program (shared by all 8 cores)."""
    T_quad = [int(TB[WPQ * (q + 1)] - TB[WPQ * q]) for q in range(NQ)]
    Tq_max = max(T_quad)

    nc = bacc.Bacc(num_devices=NCORES, num_swdge_queues=4)

    xt_in = nc.dram_tensor("xt", [128, 4, NLOC], BF, kind="ExternalInput")
    m_in = nc.dram_tensor("mt", [128, T_total * RW], BF, kind="ExternalInput")
    idx_in = nc.dram_tensor("idx", [128, T_total * 8], I16, kind="ExternalInput")
    encw_in = nc.dram_tensor("encw", [128, 4, HID], BF, kind="ExternalInput")
    encb_in = nc.dram_tensor("encb", [128, HID], F32, kind="ExternalInput")
    gcw_in = [
        nc.dram_tensor(f"gcw{l}", [HID, HID], BF, kind="ExternalInput")
        for l in range(5)
    ]
    gcb_in = [
        nc.dram_tensor(f"gcb{l}", [128, HID], F32, kind="ExternalInput")
        for l in range(5)
    ]
    attrw_in = nc.dram_tensor("attrw", [HID, FEAT], BF, kind="ExternalInput")
    attrb_in = nc.dram_tensor("attrb", [128, FEAT], F32, kind="ExternalInput")
    ident_in = nc.dram_tensor("ident", [128, 128], BF, kind="ExternalInput")
    ahat_out = nc.dram_tensor("ahat", [NLOC, N], F32, kind="ExternalOutput")
    xhat_out = nc.dram_tensor("xhat", [NLOC, FEAT], F32, kind="ExternalOutput")

    with tile.TileContext(nc) as tc:
        with (
            tc.tile_pool(name="const", bufs=1) as constp,
            tc.tile_pool(name="big", bufs=1) as bigp,
            tc.tile_pool(name="hp", bufs=44) as hp,
            tc.tile_pool(name="htp", bufs=4) as htp,
            tc.tile_pool(name="msgsp", bufs=12) as msgsp,
            tc.tile_pool(name="workp", bufs=3) as workp,
            tc.tile_pool(name="smallp", bufs=8) as smallp,
            tc.tile_pool(name="suppp", bufs=2) as suppp,
            tc.tile_pool(name="stagep", bufs=2) as stagep,
            tc.tile_pool(name="psum", bufs=1, space="PSUM") as psump,
            tc.tile_pool(name="dramp", bufs=2, space="DRAM") as dramp,
        ):
            # ---- constants ----
            ident = constp.tile([128, 128], BF)
            nc.sync.dma_start(ident[:], ident_in[:])
            encw = constp.tile([128, 4, HID], BF)
            nc.sync.dma_start(encw[:], encw_in[:])
            encb = constp.tile([128, HID], F32)
            nc.sync.dma_start(encb[:], encb_in[:])
            gcw, gcb = [], []
            for l in range(5):
                wt = constp.tile([HID, HID], BF, name=f"gcw{l}")
                nc.sync.dma_start(wt[:], gcw_in[l][:])
                gcw.append(wt)
                bt = constp.tile([128, HID], F32, name=f"gcb{l}")
                nc.sync.dma_start(bt[:], gcb_in[l][:])
                gcb.append(bt)
            attrw = constp.tile([HID, FEAT], BF)
            nc.sync.dma_start(attrw[:], attrw_in[:])
            attrb = constp.tile([128, FEAT], F32)
            nc.sync.dma_start(attrb[:], attrb_in[:])
            m_sb = constp.tile([128, T_total, RW], BF)
            nc.sync.dma_start(m_sb[:], m_in[:].rearrange("p (t r) -> p t r", r=RW))
            idx_sb = constp.tile([128, T_total * 8], I16)
            nc.sync.dma_start(idx_sb[:], idx_in[:])
            xt = bigp.tile([128, 4, NLOC], BF, tag="big")
            nc.sync.dma_start(xt[:], xt_in[:])

            # cached scalar registers for dma_gather num_idxs
            reg_cache = {}

            def gs_reg(v):
                if v not in reg_cache:
                    reg_cache[v] = nc.gpsimd.to_reg(v)
                return reg_cache[v]

            def new_hgen(nm):
                return [
                    hp.tile([128, 128], F32, name=f"h_{nm}_{j}", tag="h")
                    for j in range(NQ)
                ]

            # ---- encoder linear: h0 = x @ encW + encb (node-major) ----
            h0 = new_hgen("h0")
            hT0 = htp.tile([128, NLOC], BF, name="hT0", tag="ht")
            for j in range(NQ):
                ps = psump.tile([128, HID], F32, name="encps", tag="mm512", bufs=3)
                for k in range(4):
                    nc.tensor.matmul(
                        ps[:],
                        xt[:, k, 128 * j : 128 * (j + 1)],
                        encw[:, k, :],
                        start=(k == 0),
                        stop=(k == 3),
                    )
                nc.vector.tensor_add(h0[j][:], ps[:], encb[:])
                hb = workp.tile([128, 128], BF, name="hb0", tag="hb")
                nc.scalar.copy(hb[:], h0[j][:])
                pst = psump.tile([128, 128], BF, name="trps0", tag="tr", bufs=2)
                nc.tensor.transpose(pst[:], hb[:], ident[:])
                nc.scalar.copy(hT0[:, 128 * j : 128 * (j + 1)], pst[:])

            # ---- one GCN layer ----
            def gcn_layer(lidx, h_in, hT_in, Wt, bt, nm, keep_bf=None):
                # support = h @ W (node-major, bf16), store p-major to DRAM
                supp = suppp.tile([128, JJ, HID], BF, name=f"supp{nm}", tag="supp")
                for j in range(NQ):
                    ps = psump.tile(
                        [128, HID], F32, name=f"sps{nm}", tag="mm512", bufs=3
                    )
                    nc.tensor.matmul(
                        ps[:],
                        hT_in[:, 128 * j : 128 * (j + 1)],
                        Wt[:],
                        start=True,
                        stop=True,
                    )
                    nc.vector.tensor_copy(supp[:, j, :], ps[:])
                cc_in = dramp.tile([NLOC, HID], BF, name=f"ccin{nm}", tag="ccin")
                nc.sync.dma_start(
                    cc_in[:].rearrange("(p j) f -> p j f", p=128), supp[:]
                )
                cc_out = dramp.tile(
                    [N, HID], BF, name=f"ccout{nm}", tag="ccout", addr_space="Shared"
                )
                nc.gpsimd.collective_compute(
                    "AllGather",
                    OP.bypass,
                    replica_groups=[list(range(NCORES))],
                    ins=[cc_in.opt()],
                    outs=[cc_out.opt()],
                )

                h_out = new_hgen(nm)
                hT_out = htp.tile([128, NLOC], BF, name=f"hT{nm}", tag="ht")
                Th_max = 8
                qrr = [0]
                for q in range(NQ):
                    nT = T_quad[q]
                    bounds = [(i, min(i + 8, nT)) for i in range(0, nT, 8)]
                    msgs_l = []
                    for h0_, hn in bounds:
                        cnt = hn - h0_
                        if cnt <= 0:
                            continue
                        mt_ = msgsp.tile(
                            [128, Th_max, HID], BF, name=f"msgs{nm}", tag="msgs"
                        )
                        nc.gpsimd.dma_gather(
                            mt_[:, :cnt, :],
                            cc_out[:],
                            idx_sb[
                                :,
                                (TB[WPQ * q] + h0_) * 8 : (TB[WPQ * q] + hn) * 8,
                            ],
                            cnt * 128,
                            gs_reg(cnt * 128),
                            HID,
                            single_packet=True,
                            queue_num=qrr[0] % 4,
                        )
                        qrr[0] += 1
                        msgs_l.append((h0_, hn, mt_))

                    def msel(lt):
                        for h0_, hn, mt_ in msgs_l:
                            if h0_ <= lt < hn:
                                return mt_[:, lt - h0_, :]
                        raise AssertionError
                    ps = psump.tile([128, HID], F32, name=f"segps{nm}", tag="seg", bufs=3)
                    for wi in range(WPQ):
                        wdx = WPQ * q + wi
                        tw = int(T_w[wdx])
                        base = int(TB[wdx] - TB[WPQ * q])
                        for t in range(tw):
                            nc.tensor.matmul(
                                ps[32 * wi : 32 * (wi + 1), :],
                                m_sb[:, int(TB[wdx]) + t, :],
                                msel(base + t),
                                start=(t == 0),
                                stop=(t == tw - 1),
                                tile_position=(0, 32 * wi),
                            )
                    # finalize: out = beta*h + agg ; out /= max(||out||, eps); +b; relu
                    res = workp.tile([128, HID], F32, name=f"res{nm}", tag="res")
                    nc.vector.scalar_tensor_tensor(
                        res[:], h_in[q][:], float(BETA), ps[:], OP.mult, OP.add
                    )
                    sq = workp.tile([128, HID], F32, name=f"sq{nm}", tag="sq")
                    nsq = smallp.tile([128, 1], F32, name=f"nsq{nm}", tag="nsq")
                    nc.scalar.activation(sq[:], res[:], AF.Square, accum_out=nsq[:])
                    nrm = smallp.tile([128, 1], F32, name=f"nrm{nm}", tag="nrm")
                    nc.scalar.activation(nrm[:], nsq[:], AF.Sqrt)
                    nrm2 = smallp.tile([128, 1], F32, name=f"nrm2{nm}", tag="nrm2")
                    nc.vector.scalar_tensor_tensor(
                        nrm2[:], nrm[:], float(EPS), nrm[:], OP.max, OP.max
                    )
                    rnm = smallp.tile([128, 1], F32, name=f"rnm{nm}", tag="rnm")
                    nc.vector.reciprocal(rnm[:], nrm2[:])
                    scaled = workp.tile([128, HID], F32, name=f"scl{nm}", tag="scl")
                    nc.scalar.mul(scaled[:], res[:], rnm[:])
                    hosum = workp.tile([128, HID], F32, name=f"hos{nm}", tag="hos")
                    nc.vector.scalar_tensor_tensor(
                        hosum[:], scaled[:], 1.0, bt[:], OP.mult, OP.add
                    )
                    ho = h_out[q]
                    nc.scalar.activation(ho[:], hosum[:], AF.Relu)
                    if keep_bf is None:
                        hb = workp.tile([128, 128], BF, name=f"hb{nm}", tag="hb")
                    else:
                        hb = keep_bf[:, q, :]
                    nc.scalar.copy(hb[:], ho[:])
                    pst = psump.tile(
                        [128, 128], BF, name=f"trps{nm}", tag="tr", bufs=2
                    )
                    nc.tensor.transpose(pst[:], hb[:], ident[:])
                    nc.scalar.copy(hT_out[:, 128 * q : 128 * (q + 1)], pst[:])
                return h_out, hT_out

            h1, hT1 = gcn_layer(0, h0, hT0, gcw[0], gcb[0], "e1")
            h2, hT2 = gcn_layer(1, h1, hT1, gcw[1], gcb[1], "e2")

            # structure decoder layer; keep bf16 copy of s for the AllGather
            s_bf = suppp.tile([128, JJ, HID], BF, name="s_bf", tag="sbf", bufs=1)
            _s, sT = gcn_layer(4, h2, hT2, gcw[4], gcb[4], "s", keep_bf=s_bf)
            s_cc_in = dramp.tile([NLOC, HID], BF, name="s_ccin", tag="ccin")
            nc.sync.dma_start(
                s_cc_in[:].rearrange("(p j) f -> p j f", p=128), s_bf[:]
            )
            s_cc_out = dramp.tile(
                [N, HID], BF, name="s_ccout", tag="ccout", addr_space="Shared"
            )
            nc.gpsimd.collective_compute(
                "AllGather",
                OP.bypass,
                replica_groups=[list(range(NCORES))],
                ins=[s_cc_in.opt()],
                outs=[s_cc_out.opt()],
            )
            sfT = bigp.tile([128, N], BF, name="sfT", tag="big")
            nc.sync.dma_start(sfT[:], s_cc_out[:], transpose=True)

            # attribute decoder branch
            a1, aT1 = gcn_layer(2, h2, hT2, gcw[2], gcb[2], "a1")
            a2, aT2 = gcn_layer(3, a1, aT1, gcw[3], gcb[3], "a2")

            # x_hat = a2 @ attrW + attrb
            for j in range(NQ):
                ps = psump.tile([128, FEAT], F32, name="xps", tag="mm512", bufs=3)
                nc.tensor.matmul(
                    ps[:],
                    aT2[:, 128 * j : 128 * (j + 1)],
                    attrw[:],
                    start=True,
                    stop=True,
                )
                xst = stagep.tile([128, FEAT], F32, name="xst", tag="xst")
                nc.vector.tensor_add(xst[:], ps[:], attrb[:])
                nc.sync.dma_start(xhat_out[128 * j : 128 * (j + 1), :], xst[:])

            # a_hat = s @ s_full.T  (columns unpermuted during PSUM->SBUF copy)
            cp_flip = 0
            for j in range(NQ):
                for cb in range(8):
                    st = stagep.tile([128, NLOC], F32, name="ast", tag="ast")
                    # natural cols n = 128*jj + 32*m + pr ; st viewed [pr, jj]
                    stv = st[:].rearrange("p (jj pp) -> p pp jj", jj=JJ)
                    for m in range(4):
                        ps = psump.tile(
                            [128, 512], F32, name="aps", tag="mm512", bufs=3
                        )
                        nc.tensor.matmul(
                            ps[:],
                            sT[:, 128 * j : 128 * (j + 1)],
                            sfT[:, 2048 * cb + 512 * m : 2048 * cb + 512 * (m + 1)],
                            start=True,
                            stop=True,
                        )
                        psv = ps[:].rearrange("p (pr jj) -> p pr jj", pr=32)
                        if cp_flip % 2 == 0:
                            nc.vector.tensor_copy(
                                stv[:, 32 * m : 32 * (m + 1), :], psv
                            )
                        else:
                            nc.scalar.copy(stv[:, 32 * m : 32 * (m + 1), :], psv)
                        cp_flip += 1
                    nc.sync.dma_start(
                        ahat_out[
                            128 * j : 128 * (j + 1), 2048 * cb : 2048 * (cb + 1)
                        ],
                        st[:],
                    )

    nc.compile()
    return nc


def prep_inputs(x, prep, core):
    """Per-core input map (numpy arrays matching declared dtypes)."""
    xl = np.asarray(x[core * NLOC : (core + 1) * NLOC], np.float32)  # [2048, 512]
    xt = np.ascontiguousarray(
        xl.T.reshape(4, 128, NLOC).transpose(1, 0, 2)
    ).astype(BF_NP)
    mt = np.ascontiguousarray(
        prep["M"][core].reshape(128, prep["T_total"] * RW)
    ).astype(BF_NP)
    idx = np.ascontiguousarray(prep["idx16"][core])
    return dict(xt=xt, mt=mt, idx=idx)


def shared_inputs(enc_lin_W, enc_lin_b, enc_gc1_W, enc_gc1_b, enc_gc2_W, enc_gc2_b,
                  attr_gc1_W, attr_gc1_b, attr_gc2_W, attr_gc2_b,
                  attr_lin_W, attr_lin_b, str_gc1_W, str_gc1_b):
    def bcast(b, width):
        return np.ascontiguousarray(
            np.broadcast_to(np.asarray(b, np.float32), (128, width))
        )

    d = dict(
        encw=np.ascontiguousarray(
            np.asarray(enc_lin_W, np.float32).reshape(4, 128, HID).transpose(1, 0, 2)
        ).astype(BF_NP),
        encb=bcast(enc_lin_b, HID),
        attrw=np.asarray(attr_lin_W, np.float32).astype(BF_NP),
        attrb=bcast(attr_lin_b, FEAT),
        ident=np.eye(128, dtype=BF_NP),
    )
    gws = [enc_gc1_W, enc_gc2_W, attr_gc1_W, attr_gc2_W, str_gc1_W]
    gbs = [enc_gc1_b, enc_gc2_b, attr_gc1_b, attr_gc2_b, str_gc1_b]
    for l in range(5):
        d[f"gcw{l}"] = np.asarray(gws[l], np.float32).astype(BF_NP)
        d[f"gcb{l}"] = bcast(gbs[l], HID)
    return d


_BUILD_CACHE = {}


def kernel(x, edge_index, edge_weight,
           enc_lin_W, enc_lin_b, enc_gc1_W, enc_gc1_b, enc_gc2_W, enc_gc2_b,
           attr_gc1_W, attr_gc1_b, attr_gc2_W, attr_gc2_b, attr_lin_W, attr_lin_b,
           str_gc1_W, str_gc1_b, _trace=False):
    x = np.asarray(x, np.float32)
    edge_index = np.asarray(edge_index)
    edge_weight = np.asarray(edge_weight, np.float32)

    prep = prep_graph(edge_index, edge_weight)
    ck = prep["M"].tobytes()[:0]  # program depends only on T_w/TB
    key = (tuple(prep["T_w"].tolist()),)
    if key not in _BUILD_CACHE:
        _BUILD_CACHE[key] = build_program(prep["T_w"], prep["TB"], prep["T_total"])
    nc = _BUILD_CACHE[key]

    shared = shared_inputs(
        enc_lin_W, enc_lin_b, enc_gc1_W, enc_gc1_b, enc_gc2_W, enc_gc2_b,
        attr_gc1_W, attr_gc1_b, attr_gc2_W, attr_gc2_b, attr_lin_W, attr_lin_b,
        str_gc1_W, str_gc1_b,
    )
    in_maps = []
    for c in range(NCORES):
        m = prep_inputs(x, prep, c)
        m.update(shared)
        in_maps.append(m)

    res = run_bass_kernel_spmd(nc, in_maps, list(range(NCORES)), trace=_trace)

    a_hat = np.concatenate([res.results[c]["ahat"] for c in range(NCORES)], axis=0)
    x_hat = np.concatenate([res.results[c]["xhat"] for c in range(NCORES)], axis=0)
    if _trace:
        kernel.last_exec_time_ns = res.exec_time_ns
        kernel.last_results = res
    return (a_hat, x_hat)
